# revision 24
# baseline (speedup 1.0000x reference)
"""MLA (multi-head latent attention) prefill block on 8 Trainium2 NeuronCores.

Tensor-parallel over heads: each core computes 4 of the 32 heads end-to-end.
Unlike the absorbed (decode-style) formulation, this kernel materializes
per-head K = kv_c @ wbk^T [S, 128] and V = kv_c @ wbv [S, 128] explicitly,
so scores contract over 192 dims (128 nope + 64 rope) instead of 576 and
the attention output contracts over 128 instead of 512 — ~1.6x fewer MACs.

The kv_a projection + rms-norm + rope (otherwise replicated on all 8 cores)
is sharded over the sequence: each core computes 2 of the 16 kv tiles,
transposes them, and an AllGather collective distributes the transposed
latents while the PE runs the q projections.

All matmul operands are bf16 (1 cycle/row on the PE, same as f32r, but half
the SBUF/DMA traffic); softmax statistics and rope stay f32. Per-core
partial outputs of the row-parallel wo matmul are summed on the host.

Self-contained: hardcodes all shapes from the problem spec.
"""

import os
from contextlib import ExitStack

import numpy as np

import concourse.bacc as bacc
import concourse.bass as bass
import concourse.mybir as mybir
import concourse.tile as tile
from concourse.bass_utils import run_bass_kernel_spmd
from concourse.masks import make_identity

# ---- problem constants ----
DIM = 2048
NH = 32
DN = 128  # qk_nope_head_dim
DR = 64   # qk_rope_head_dim
DV = 128  # v_head_dim
KVL = 512  # kv_lora_rank
S = 2048   # sequence length (B=1)
SCALE = float((DN + DR) ** -0.5)
EPS = 1e-6

NCORES = 8
NHC = NH // NCORES      # heads per core = 4
P = 128                 # partitions
SF = 512                # free-dim tile (s tiles)
NST = S // SF           # 4 s tiles
NTT = S // P            # 16 t tiles
NDC = DIM // P          # 16 contraction chunks over model dim
NCC = KVL // P          # 4 latent chunks

F32 = mybir.dt.float32
BF16 = mybir.dt.bfloat16
F16 = mybir.dt.float16
RT = BF16  # dtype for all matmul operands

# Shard the kv_a projection across cores + AllGather (vs replicate)
SHARD_KV = os.environ.get("MLA_SHARD_KV", "1") == "1"
MSP_BUFS = int(os.environ.get("MLA_MSP_BUFS", "4"))
OHP_BUFS = int(os.environ.get("MLA_OHP_BUFS", "2"))
ETP_BUFS = int(os.environ.get("MLA_ETP_BUFS", "6"))
TSH = S // NCORES  # 256 seq positions (2 t tiles) owned per core


def build_nc(repeat=1):
    """Build the per-core Bass program (identical on all 8 cores)."""
    nc = bacc.Bacc("TRN2", target_bir_lowering=False, debug=False,
                   num_devices=NCORES)

    # ---- DRAM I/O ----
    d_xT = nc.dram_tensor("xT", [DIM, S], RT, kind="ExternalInput")
    d_wqn = nc.dram_tensor("wq_n", [DIM, NHC * DN], RT, kind="ExternalInput")
    d_wqpr = nc.dram_tensor("wq_pr", [DIM, NHC * 32], RT, kind="ExternalInput")
    d_wqpi = nc.dram_tensor("wq_pi", [DIM, NHC * 32], RT, kind="ExternalInput")
    d_wkva = nc.dram_tensor("wkv_a", [DIM, KVL + DR], RT, kind="ExternalInput")
    d_wbkT = nc.dram_tensor("wbkT", [NHC, KVL, DN], RT, kind="ExternalInput")
    d_wbv = nc.dram_tensor("wbv_all", [KVL, NHC * DV], RT,
                           kind="ExternalInput")
    d_wo = nc.dram_tensor("wo_c", [NHC * DV, DIM], RT, kind="ExternalInput")
    d_cosr = nc.dram_tensor("cosR", [P, S], F32, kind="ExternalInput")
    d_sinr = nc.dram_tensor("sinR", [P, S], F32, kind="ExternalInput")
    if SHARD_KV:
        d_xo = nc.dram_tensor("x_own", [DIM, TSH], RT, kind="ExternalInput")
        d_coso = nc.dram_tensor("cos_o", [P, 2 * 32], F32,
                                kind="ExternalInput")
        d_sino = nc.dram_tensor("sin_o", [P, 2 * 32], F32,
                                kind="ExternalInput")
    else:
        d_cosn = nc.dram_tensor("cos_n", [S, DR // 2], F32,
                                kind="ExternalInput")
        d_sinn = nc.dram_tensor("sin_n", [S, DR // 2], F32,
                                kind="ExternalInput")
    d_out = nc.dram_tensor("outT", [DIM, S], F16, kind="ExternalOutput")

    out = d_out.ap()

    with tile.TileContext(nc) as tc:
      for _rep in range(repeat):
        with ExitStack() as top:
            cst = top.enter_context(tc.tile_pool(name="const", bufs=1))
            ident = cst.tile([P, P], RT, tag="ident", name="ident")
            make_identity(nc, ident[:])
            ones_c = cst.tile([P, 1], RT, tag="ones_c", name="ones_c")
            nc.gpsimd.memset(ones_c[:], 1.0)
            epsb = cst.tile([P, 1], F32, tag="epsb", name="epsb")
            nc.gpsimd.memset(epsb[:], EPS)

            # transposed latents, shared by all heads
            kvtp = top.enter_context(tc.tile_pool(name="kvT", bufs=NCC))
            kptp = top.enter_context(tc.tile_pool(name="kpT", bufs=1))
            kvcT = [kvtp.tile([P, S], RT, tag="kvcT", name="kvcT")
                    for _ in range(NCC)]
            kpeT = kptp.tile([DR, S], RT, tag="kpeT", name="kpeT")
            # q for all 4 heads, kept in SBUF
            qnp = top.enter_context(tc.tile_pool(name="qn", bufs=NHC))
            qns = [qnp.tile([DN, S], RT, tag="qn", name="qn")
                   for _ in range(NHC)]
            qpp = top.enter_context(tc.tile_pool(name="qp", bufs=NHC))
            qps = [qpp.tile([DR, S], RT, tag="qp", name="qp")
                   for _ in range(NHC)]
            wkp = top.enter_context(tc.tile_pool(name="wkva", bufs=1))
            wkva_a = wkp.tile([P, NDC * (KVL + DR)], RT, tag="wkva",
                              name="wkva")

            # ===== phase 0: kv shard (2 t-tiles) + AllGather ===============
            if SHARD_KV:
              with ExitStack() as ph0:
                p0s = ph0.enter_context(tc.tile_pool(name="p0s", bufs=1))
                xo = p0s.tile([P, NDC * TSH], RT, tag="xo", name="xo")
                # interleave x-shard and wkva quarters in consumption order
                for q4 in range(4):
                    hd = slice(q4 * (NDC // 4), (q4 + 1) * (NDC // 4))
                    nc.sync.dma_start(
                        xo[:].rearrange("p (d f) -> p d f", d=NDC)[:, hd],
                        d_xo.ap().rearrange("(d p) f -> p d f", p=P)[:, hd])
                    nc.sync.dma_start(
                        wkva_a[:].rearrange("p (d c) -> p d c",
                                            d=NDC)[:, hd],
                        d_wkva.ap().rearrange("(d p) c -> p d c",
                                              p=P)[:, hd])
                coso = p0s.tile([P, 2 * 32], F32, tag="coso", name="coso")
                sino = p0s.tile([P, 2 * 32], F32, tag="sino", name="sino")
                nc.sync.dma_start(coso[:], d_coso.ap())
                nc.sync.dma_start(sino[:], d_sino.ap())
                shT = p0s.tile([P, 5 * TSH], RT, tag="shT", name="shT")
                kvo = [p0s.tile([P, KVL], RT, tag="kvo", name="kvo")
                       for _ in range(2)]
                kpo = [p0s.tile([P, DR], RT, tag="kpo", name="kpo")
                       for _ in range(2)]
                nrm0 = ph0.enter_context(tc.tile_pool(name="nrm0", bufs=2))
                with tc.tile_pool(name="p0a", bufs=4, space="PSUM") as p0a:
                    for ti in range(2):
                        psc = p0a.tile([P, KVL], F32, tag="acc", name="acc")
                        psp = p0a.tile([P, DR], F32, tag="acc",
                                       name="accp", padded_shape=[P, KVL])
                        for d in range(NDC):
                            xtsl = xo[:, d * TSH + ti * P:
                                      d * TSH + (ti + 1) * P]
                            nc.tensor.matmul(
                                psc[:], xtsl,
                                wkva_a[:, d * (KVL + DR):
                                       d * (KVL + DR) + KVL],
                                start=(d == 0), stop=(d == NDC - 1))
                            nc.tensor.matmul(
                                psp[:], xtsl,
                                wkva_a[:, d * (KVL + DR) + KVL:
                                       (d + 1) * (KVL + DR)],
                                start=(d == 0), stop=(d == NDC - 1))
                        sq = nrm0.tile([P, KVL], F32, tag="sq", name="sq")
                        ss = nrm0.tile([P, 1], F32, tag="ss", name="ss")
                        nc.scalar.activation(
                            sq[:], psc[:],
                            mybir.ActivationFunctionType.Square,
                            accum_out=ss[:])
                        rt_ = nrm0.tile([P, 1], F32, tag="rt", name="rt")
                        nc.scalar.activation(
                            rt_[:], ss[:],
                            mybir.ActivationFunctionType.Sqrt,
                            bias=epsb[:], scale=1.0 / KVL)
                        ri = nrm0.tile([P, 1], F32, tag="ri", name="ri")
                        nc.vector.reciprocal(ri[:], rt_[:])
                        nc.scalar.mul(kvo[ti][:], psc[:], ri[:])
                        # k rope (deinterleave to [r(32) | i(32)])
                        cn = coso[:, ti * 32:(ti + 1) * 32]
                        sn = sino[:, ti * 32:(ti + 1) * 32]
                        pe = psp[:].rearrange("p (k two) -> p k two", two=2)
                        xr = pe[:, :, 0:1].rearrange("p k one -> p (k one)")
                        xi = pe[:, :, 1:2].rearrange("p k one -> p (k one)")
                        m1 = nrm0.tile([P, DR // 2], F32, tag="m1", name="m1")
                        m2 = nrm0.tile([P, DR // 2], F32, tag="m2", name="m2")
                        nc.vector.tensor_mul(m1[:], xr, cn)
                        nc.vector.tensor_mul(m2[:], xi, sn)
                        nc.vector.tensor_sub(kpo[ti][:, 0:32], m1[:], m2[:])
                        nc.vector.tensor_mul(m1[:], xr, sn)
                        nc.vector.tensor_mul(m2[:], xi, cn)
                        nc.vector.tensor_add(kpo[ti][:, 32:64], m1[:], m2[:])
                    # transpose own tiles into shard layout
                    with tc.tile_pool(name="tp0", bufs=4,
                                      space="PSUM") as tp0:
                        for ti in range(2):
                            for cc in range(NCC):
                                tp = tp0.tile([P, P], RT, tag="t", name="t")
                                nc.tensor.transpose(
                                    tp[:],
                                    kvo[ti][:, cc * P:(cc + 1) * P],
                                    ident[:])
                                nc.scalar.copy(
                                    shT[:, cc * TSH + ti * P:
                                        cc * TSH + (ti + 1) * P], tp[:])
                            tp = tp0.tile([P, P], RT, tag="t", name="t")
                            nc.tensor.transpose(tp[0:DR, :], kpo[ti][:],
                                                ident[:])
                            nc.scalar.copy(
                                shT[0:DR, NCC * TSH + ti * P:
                                    NCC * TSH + (ti + 1) * P], tp[0:DR, :])
                # ---- AllGather the transposed shard ----
                drp = top.enter_context(
                    tc.tile_pool(name="dram", bufs=1, space="DRAM"))
                bin_ = drp.tile([KVL + DR, TSH], RT, tag="cc_in",
                                name="cc_in")
                bout = drp.tile([NCORES, KVL + DR, TSH], RT, tag="cc_out",
                                name="cc_out", addr_space="Shared")
                nc.gpsimd.dma_start(
                    bin_[0:KVL, :].rearrange("(cc p) f -> p cc f", p=P),
                    shT[:].rearrange("p (b f) -> p b f", b=5)[:, 0:NCC])
                nc.gpsimd.dma_start(
                    bin_[KVL:KVL + DR, :],
                    shT[0:DR, NCC * TSH:5 * TSH])
                nc.gpsimd.collective_compute(
                    "AllGather", mybir.AluOpType.bypass,
                    replica_groups=[list(range(NCORES))],
                    ins=[bin_[:].opt()],
                    outs=[bout[:].opt()])
                for cc in range(NCC):
                    nc.gpsimd.dma_start(
                        kvcT[cc][:].rearrange("p (g f) -> p g f", g=NCORES),
                        bout[:, cc * P:(cc + 1) * P, :].rearrange(
                            "g p f -> p g f"))
                nc.gpsimd.dma_start(
                    kpeT[:].rearrange("p (g f) -> p g f", g=NCORES),
                    bout[:, KVL:KVL + DR, :].rearrange("g p f -> p g f"))

            # ===== phase 1: q (+ kv if not sharded) projections ============
            kvc = kpe = None
            if not SHARD_KV:
                for q4 in range(4):
                    hd = slice(q4 * (NDC // 4), (q4 + 1) * (NDC // 4))
                    nc.sync.dma_start(
                        wkva_a[:].rearrange("p (d c) -> p d c",
                                            d=NDC)[:, hd],
                        d_wkva.ap().rearrange("(d p) c -> p d c",
                                              p=P)[:, hd])
                kvp = top.enter_context(tc.tile_pool(name="kv", bufs=NTT))
                kvc = [kvp.tile([P, KVL], RT, tag="kvc", name="kvc")
                       for _ in range(NTT)]
                kpp = top.enter_context(tc.tile_pool(name="kpe", bufs=NTT))
                kpe = [kpp.tile([P, DR], RT, tag="kpe", name="kpe")
                       for _ in range(NTT)]

            with ExitStack() as ph1:
                wrp = ph1.enter_context(tc.tile_pool(name="wres", bufs=1))
                xsl = ph1.enter_context(tc.tile_pool(name="xsl", bufs=6))
                xTj0 = d_xT.ap()[:, 0:SF].rearrange("(d p) f -> p d f", p=P)
                xh0 = [xsl.tile([P, 4 * SF], RT, tag="xsl", name="xsl")
                       for _ in range(4)]
                wqn_a = wrp.tile([P, NDC * NHC * DN], RT, tag="wqn",
                                 name="wqn")
                wqpr_a = wrp.tile([P, NDC * NHC * 32], RT, tag="wqpr",
                                  name="wqpr")
                wqpi_a = wrp.tile([P, NDC * NHC * 32], RT, tag="wqpi",
                                  name="wqpi")
                for q4 in range(4):
                    hd = slice(q4 * (NDC // 4), (q4 + 1) * (NDC // 4))
                    nc.sync.dma_start(
                        xh0[q4][:].rearrange("p (d f) -> p d f", d=4),
                        xTj0[:, 4 * q4:4 * (q4 + 1)])
                    nc.sync.dma_start(
                        wqn_a[:].rearrange("p (d c) -> p d c", d=NDC)[:, hd],
                        d_wqn.ap().rearrange("(d p) c -> p d c", p=P)[:, hd])
                    nc.sync.dma_start(
                        wqpr_a[:].rearrange("p (d c) -> p d c", d=NDC)[:, hd],
                        d_wqpr.ap().rearrange("(d p) c -> p d c", p=P)[:, hd])
                    nc.sync.dma_start(
                        wqpi_a[:].rearrange("p (d c) -> p d c", d=NDC)[:, hd],
                        d_wqpi.ap().rearrange("(d p) c -> p d c", p=P)[:, hd])
                if not SHARD_KV:
                    cna = wrp.tile([P, NTT * 32], F32, tag="cna", name="cna")
                    sna = wrp.tile([P, NTT * 32], F32, tag="sna", name="sna")
                    nc.sync.dma_start(
                        cna[:].rearrange("p (t k) -> p t k", t=NTT),
                        d_cosn.ap().rearrange("(t p) k -> p t k", p=P))
                    nc.sync.dma_start(
                        sna[:].rearrange("p (t k) -> p t k", t=NTT),
                        d_sinn.ap().rearrange("(t p) k -> p t k", p=P))

                rts = ph1.enter_context(tc.tile_pool(name="ropetmp", bufs=1))
                sqs = ph1.enter_context(tc.tile_pool(name="sqs", bufs=2))
                crs = ph1.enter_context(tc.tile_pool(name="crs", bufs=2))
                kct = ph1.enter_context(tc.tile_pool(name="kct", bufs=2))
                nrm = ph1.enter_context(tc.tile_pool(name="nrm", bufs=4))

                with tc.tile_pool(name="acc1", bufs=8, space="PSUM") as qac:
                    for j in range(NST):
                        js = slice(j * SF, (j + 1) * SF)
                        xTj = d_xT.ap()[:, js].rearrange(
                            "(d p) f -> p d f", p=P)
                        if j == 0:
                            xh = xh0
                        else:
                            xh = [xsl.tile([P, 4 * SF], RT, tag="xsl",
                                           name="xsl") for _ in range(4)]
                            for q4 in range(4):
                                nc.sync.dma_start(
                                    xh[q4][:].rearrange(
                                        "p (d f) -> p d f", d=4),
                                    xTj[:, 4 * q4:4 * (q4 + 1)])
                        # ---- q projections for this s block ----
                        pss = [qac.tile([P, SF], F32, tag="acc", name="acc")
                               for _ in range(NHC + 2)]
                        for d in range(NDC):
                            xs = xh[d // 4][:, (d % 4) * SF:(d % 4 + 1) * SF]
                            for h in range(NHC):
                                nc.tensor.matmul(
                                    pss[h][:],
                                    wqn_a[:, d * NHC * DN + h * DN:
                                          d * NHC * DN + (h + 1) * DN],
                                    xs,
                                    start=(d == 0), stop=(d == NDC - 1))
                            nc.tensor.matmul(
                                pss[NHC][:],
                                wqpr_a[:, d * P:(d + 1) * P], xs,
                                start=(d == 0), stop=(d == NDC - 1))
                            nc.tensor.matmul(
                                pss[NHC + 1][:],
                                wqpi_a[:, d * P:(d + 1) * P], xs,
                                start=(d == 0), stop=(d == NDC - 1))
                        for h in range(NHC):
                            nc.scalar.copy(qns[h][:, js], pss[h][:])
                        # rope rotation for q_pe (even=r, odd=i) off PSUM
                        t1 = rts.tile([P, SF], F32, tag="t1", name="t1")
                        t2 = rts.tile([P, SF], F32, tag="t2", name="t2")
                        ror = rts.tile([P, SF], F32, tag="ror", name="ror")
                        roi = rts.tile([P, SF], F32, tag="roi", name="roi")
                        cR = crs.tile([P, SF], F32, tag="cR", name="cR")
                        sR = crs.tile([P, SF], F32, tag="sR", name="sR")
                        nc.sync.dma_start(cR[:], d_cosr.ap()[:, js])
                        nc.sync.dma_start(sR[:], d_sinr.ap()[:, js])
                        nc.vector.tensor_mul(t1[:], pss[NHC][:], cR[:])
                        nc.vector.tensor_mul(t2[:], pss[NHC + 1][:], sR[:])
                        nc.vector.tensor_sub(ror[:], t1[:], t2[:])
                        nc.vector.tensor_mul(t1[:], pss[NHC][:], sR[:])
                        nc.vector.tensor_mul(t2[:], pss[NHC + 1][:], cR[:])
                        nc.vector.tensor_add(roi[:], t1[:], t2[:])
                        for h in range(NHC):
                            hs = slice(h * 32, (h + 1) * 32)
                            nc.vector.tensor_copy(
                                qps[h][0:32, js], ror[hs, :])
                            nc.vector.tensor_copy(
                                qps[h][32:64, js], roi[hs, :])
                        if SHARD_KV:
                            continue
                        # ---- kv projection for this t block (same x) ----
                        for ti in range(4):
                            psc = qac.tile([P, KVL], F32, tag="acc",
                                           name="acc")
                            psp = qac.tile([P, DR], F32, tag="acc",
                                           name="accp", padded_shape=[P, SF])
                            for d in range(NDC):
                                xs = xh[d // 4]
                                xtsl = xs[:, (d % 4) * SF + ti * P:
                                          (d % 4) * SF + (ti + 1) * P]
                                nc.tensor.matmul(
                                    psc[:], xtsl,
                                    wkva_a[:, d * (KVL + DR):
                                           d * (KVL + DR) + KVL],
                                    start=(d == 0), stop=(d == NDC - 1))
                                nc.tensor.matmul(
                                    psp[:], xtsl,
                                    wkva_a[:, d * (KVL + DR) + KVL:
                                           (d + 1) * (KVL + DR)],
                                    start=(d == 0), stop=(d == NDC - 1))
                            t = j * 4 + ti
                            sq = sqs.tile([P, KVL], F32, tag="sq", name="sq")
                            ss = nrm.tile([P, 1], F32, tag="ss", name="ss")
                            nc.scalar.activation(
                                sq[:], psc[:],
                                mybir.ActivationFunctionType.Square,
                                accum_out=ss[:])
                            rt_ = nrm.tile([P, 1], F32, tag="rt", name="rt")
                            nc.scalar.activation(
                                rt_[:], ss[:],
                                mybir.ActivationFunctionType.Sqrt,
                                bias=epsb[:], scale=1.0 / KVL)
                            ri = nrm.tile([P, 1], F32, tag="ri", name="ri")
                            nc.vector.reciprocal(ri[:], rt_[:])
                            nc.scalar.mul(kvc[t][:], psc[:], ri[:])
                            # k rope (deinterleave to [r(32) | i(32)])
                            cn = cna[:, t * 32:(t + 1) * 32]
                            sn = sna[:, t * 32:(t + 1) * 32]
                            pe = psp[:].rearrange(
                                "p (k two) -> p k two", two=2)
                            xr = pe[:, :, 0:1].rearrange(
                                "p k one -> p (k one)")
                            xi = pe[:, :, 1:2].rearrange(
                                "p k one -> p (k one)")
                            m1 = kct.tile([P, DR // 2], F32, tag="m1",
                                          name="m1")
                            m2 = kct.tile([P, DR // 2], F32, tag="m2",
                                          name="m2")
                            nc.vector.tensor_mul(m1[:], xr, cn)
                            nc.vector.tensor_mul(m2[:], xi, sn)
                            nc.vector.tensor_sub(kpe[t][:, 0:32], m1[:],
                                                 m2[:])
                            nc.vector.tensor_mul(m1[:], xr, sn)
                            nc.vector.tensor_mul(m2[:], xi, cn)
                            nc.vector.tensor_add(kpe[t][:, 32:64], m1[:],
                                                 m2[:])

            # ======== phase 2: transposes, K/V materialize, attention ======
            msp = top.enter_context(
                tc.tile_pool(name="msp", bufs=MSP_BUFS, space="PSUM"))
            otp = top.enter_context(tc.tile_pool(name="oT", bufs=NHC))
            oTs = [otp.tile([DV, S], RT, tag="oT", name="oT")
                   for _ in range(NHC)]
            if not SHARD_KV:
                with tc.tile_pool(name="tpp", bufs=4, space="PSUM") as tpp:
                    for t in range(NTT):
                        ts_ = slice(t * P, (t + 1) * P)
                        for cc in range(NCC):
                            tp = tpp.tile([P, P], RT, tag="mspt",
                                          name="mspt")
                            nc.tensor.transpose(
                                tp[:], kvc[t][:, cc * P:(cc + 1) * P],
                                ident[:])
                            nc.scalar.copy(kvcT[cc][:, ts_], tp[:])
                        tp = tpp.tile([P, P], RT, tag="mspt", name="mspt")
                        nc.tensor.transpose(tp[0:DR, :], kpe[t][:], ident[:])
                        nc.scalar.copy(kpeT[:, ts_], tp[0:DR, :])

            # V for all 4 heads: V_all[t] = kv_c[t] @ wbv  -> [t(128), 4*DV]
            vap = top.enter_context(tc.tile_pool(name="vall", bufs=NTT))
            wbvp = top.enter_context(tc.tile_pool(name="wbv", bufs=1))
            wbv_a = wbvp.tile([P, NCC * NHC * DV], RT, tag="wbv",
                              name="wbv")
            nc.sync.dma_start(
                wbv_a[:].rearrange("p (cc f) -> p cc f", cc=NCC),
                d_wbv.ap().rearrange("(cc p) f -> p cc f", p=P))
            vall = [vap.tile([P, NHC * DV], RT, tag="vall", name="vall")
                    for _ in range(NTT)]
            for t in range(NTT):
                ts_ = slice(t * P, (t + 1) * P)
                ps = msp.tile([P, SF], F32, tag="msp", name="msp")
                for cc in range(NCC):
                    nc.tensor.matmul(
                        ps[:], kvcT[cc][:, ts_],
                        wbv_a[:, cc * NHC * DV:(cc + 1) * NHC * DV],
                        start=(cc == 0), stop=(cc == NCC - 1))
                nc.scalar.copy(vall[t][:], ps[:])

            with ExitStack() as ph2:
                wbkp = ph2.enter_context(tc.tile_pool(name="wbk", bufs=2))
                ktp = ph2.enter_context(tc.tile_pool(name="kt", bufs=NHC))
                etp = ph2.enter_context(tc.tile_pool(name="et", bufs=ETP_BUFS))
                ohp = ph2.enter_context(
                    tc.tile_pool(name="ohp", bufs=OHP_BUFS, space="PSUM"))
                dnp = ph2.enter_context(
                    tc.tile_pool(name="dn", bufs=2, space="PSUM"))
                dvp = ph2.enter_context(tc.tile_pool(name="dinv", bufs=2))
                wop = ph2.enter_context(tc.tile_pool(name="wo", bufs=NHC))
                otg = ph2.enter_context(tc.tile_pool(name="ost", bufs=3))

                # prefetch wo weights early (consumed by the per-j output
                # projection below)
                wos = [wop.tile([DV, DIM], RT, tag="wo", name="wo")
                       for _ in range(NHC)]
                for h in range(NHC):
                    nc.sync.dma_start(
                        wos[h][:], d_wo.ap()[h * DV:(h + 1) * DV, :])

                # K_h^T = wbk_h^T(scaled) @ kv_c^T   [DN, S], all heads
                kts = [ktp.tile([DN, S], RT, tag="kT", name="kT")
                       for _ in range(NHC)]
                for h in range(NHC):
                    wbk = wbkp.tile([P, NCC * DN], RT, tag="wbk", name="wbk")
                    nc.sync.dma_start(
                        wbk[:].rearrange("p (cc f) -> p cc f", cc=NCC),
                        d_wbkT.ap()[h].rearrange("(cc p) f -> p cc f", p=P))
                    for j in range(NST):
                        js = slice(j * SF, (j + 1) * SF)
                        ps = msp.tile([P, SF], F32, tag="msp", name="msp")
                        for cc in range(NCC):
                            nc.tensor.matmul(
                                ps[:], wbk[:, cc * DN:(cc + 1) * DN],
                                kvcT[cc][:, js],
                                start=(cc == 0), stop=(cc == NCC - 1))
                        nc.scalar.copy(kts[h][:, js], ps[:])

                # attention j-outer / h-inner (heads interleave to hide the
                # exp->mask latency on diagonal tiles), then the output
                # projection for this j right away
                for j in range(NST):
                    js = slice(j * SF, (j + 1) * SF)
                    for h in range(NHC):
                        oh = ohp.tile([P, SF], F32, tag="oh", name="oh")
                        dn = dnp.tile([1, SF], F32, tag="dn", name="dn")
                        ntt = 4 * j + 4
                        for t in range(ntt):
                            ts_ = slice(t * P, (t + 1) * P)
                            # causal narrowing: tile t only needs columns
                            # s >= t*128, i.e. local offset 128*(t-4j)
                            off = max(0, P * (t - 4 * j))
                            nf = SF - off
                            osl = slice(j * SF + off, (j + 1) * SF)
                            sc = msp.tile([P, SF], F32, tag="msp",
                                          name="msp")
                            nc.tensor.matmul(
                                sc[:, 0:nf], kts[h][:, ts_], qns[h][:, osl],
                                start=True, stop=False)
                            nc.tensor.matmul(
                                sc[:, 0:nf], kpeT[:, ts_], qps[h][:, osl],
                                start=False, stop=True)
                            e = etp.tile([P, SF], RT, tag="et", name="et")
                            nc.scalar.activation(
                                e[:, 0:nf], sc[:, 0:nf],
                                mybir.ActivationFunctionType.Exp,
                                scale=SCALE)
                            if t >= 4 * j:
                                # triangular mask on the first 128 columns
                                # of the diagonal tile (keep c >= p)
                                nc.gpsimd.affine_select(
                                    out=e[:, 0:P], in_=e[:, 0:P],
                                    compare_op=mybir.AluOpType.is_ge,
                                    fill=0.0, base=0,
                                    pattern=[[1, P]],
                                    channel_multiplier=-1)
                            nc.tensor.matmul(
                                dn[:, off:SF], ones_c[:], e[:, 0:nf],
                                start=(t == 0), stop=(t == ntt - 1))
                            nc.tensor.matmul(
                                oh[:, off:SF],
                                vall[t][:, h * DV:(h + 1) * DV],
                                e[:, 0:nf], start=(t == 0),
                                stop=(t == ntt - 1))
                        di = dvp.tile([1, SF], RT, tag="di", name="di")
                        with nc.allow_low_precision(
                                reason="bf16 1/denom is within tolerance"):
                            nc.vector.reciprocal(di[:], dn[:])
                        db = dvp.tile([P, SF], RT, tag="db", name="db")
                        nc.gpsimd.partition_broadcast(db[:], di[:])
                        nc.vector.tensor_mul(oTs[h][:, js], oh[:], db[:])
                    # ---- output projection for this s block ----
                    for d in range(NDC):
                        ds_ = slice(d * P, (d + 1) * P)
                        ps = msp.tile([P, SF], F32, tag="msp", name="msp")
                        for h in range(NHC):
                            nc.tensor.matmul(
                                ps[:], wos[h][:, ds_], oTs[h][:, js],
                                start=(h == 0), stop=(h == NHC - 1))
                        obig = otg.tile([P, SF], F16, tag="ost", name="ost")
                        nc.vector.tensor_copy(obig[:], ps[:])
                        # out DMAs go on the ACT hwdge queue: they wait on
                        # compute, and would head-of-line block the next
                        # rep's input loads on the SP queue
                        nc.scalar.dma_start(out[ds_, js], obig[:])

    nc.compile()
    return nc


def prep_inputs(x, wq_w, wkv_a_w, wkv_b_w, kv_norm_w, wo_w,
                freqs_cos, freqs_sin):
    """Host-side sharding/layout prep. Returns per-core input maps."""
    import ml_dtypes
    bf16 = ml_dtypes.bfloat16
    x = np.ascontiguousarray(np.asarray(x, np.float32).reshape(S, DIM))
    xT = np.ascontiguousarray(x.T).astype(bf16)
    wq = np.asarray(wq_w, np.float32).reshape(DIM, NH, DN + DR)
    wkva = np.ascontiguousarray(np.asarray(wkv_a_w, np.float32)).astype(bf16)
    wkvb = np.asarray(wkv_b_w, np.float32)
    knw = np.asarray(kv_norm_w, np.float32)
    wo = np.asarray(wo_w, np.float32)
    cos = np.asarray(freqs_cos, np.float32)
    sin = np.asarray(freqs_sin, np.float32)
    cosR = np.ascontiguousarray(np.tile(cos.T, (NHC, 1)))  # [128, S]
    sinR = np.ascontiguousarray(np.tile(sin.T, (NHC, 1)))

    maps = []
    for c in range(NCORES):
        hs = list(range(NHC * c, NHC * (c + 1)))
        wq_n = np.ascontiguousarray(
            wq[:, hs, :DN].reshape(DIM, NHC * DN)).astype(bf16)
        wq_pr = np.ascontiguousarray(
            wq[:, hs, DN + 0::2].reshape(DIM, NHC * 32)).astype(bf16)
        wq_pi = np.ascontiguousarray(
            wq[:, hs, DN + 1::2].reshape(DIM, NHC * 32)).astype(bf16)
        # fold kv_norm weight into the absorbed weights
        # wbkT[h] = (wbk_h * knw).T  [KVL, DN]
        wbkT = np.stack([
            np.ascontiguousarray(
                (wkvb[h * (DN + DV):h * (DN + DV) + DN, :] * knw[None, :]).T)
            for h in hs]).astype(bf16)                      # [4, 512, 128]
        # wbv_all = concat_h (wbv_h^T * knw[:,None])  [KVL, 4*DV]
        wbv_all = np.concatenate(
            [np.ascontiguousarray(
                wkvb[h * (DN + DV) + DN:(h + 1) * (DN + DV), :].T)
             * knw[:, None] for h in hs], axis=1).astype(bf16)  # [512, 512]
        wo_c = np.ascontiguousarray(
            np.concatenate([wo[h * DV:(h + 1) * DV, :]
                            for h in hs])).astype(bf16)
        m = {
            "xT": xT, "wq_n": wq_n, "wq_pr": wq_pr, "wq_pi": wq_pi,
            "wkv_a": wkva, "wbkT": np.ascontiguousarray(wbkT),
            "wbv_all": np.ascontiguousarray(wbv_all), "wo_c": wo_c,
            "cosR": cosR, "sinR": sinR,
        }
        if SHARD_KV:
            m["x_own"] = np.ascontiguousarray(
                xT[:, c * TSH:(c + 1) * TSH])
            # cos/sin for own 2 t-tiles, [128, 2*32]
            co = cos[c * TSH:(c + 1) * TSH].reshape(2, P, DR // 2)
            si = sin[c * TSH:(c + 1) * TSH].reshape(2, P, DR // 2)
            m["cos_o"] = np.ascontiguousarray(
                co.transpose(1, 0, 2).reshape(P, 2 * 32))
            m["sin_o"] = np.ascontiguousarray(
                si.transpose(1, 0, 2).reshape(P, 2 * 32))
        else:
            m["cos_n"] = cos
            m["sin_n"] = sin
        maps.append(m)
    return maps


def kernel(x, wq_w, wkv_a_w, wkv_b_w, kv_norm_w, wo_w,
           freqs_cos, freqs_sin, start_pos):
    assert int(start_pos) == 0
    maps = prep_inputs(x, wq_w, wkv_a_w, wkv_b_w, kv_norm_w, wo_w,
                       freqs_cos, freqs_sin)
    nc = build_nc()
    res = run_bass_kernel_spmd(nc, maps, list(range(NCORES)))
    acc = np.zeros((DIM, S), np.float64)
    for c in range(NCORES):
        acc += res.results[c]["outT"].astype(np.float64)
    return np.ascontiguousarray(acc.T).astype(np.float32).reshape(1, S, DIM)


# revision 25
# speedup vs baseline: 1.0914x; 1.0914x over previous
"""MLA (multi-head latent attention) prefill block on 8 Trainium2 NeuronCores.

Tensor-parallel over heads: each core computes 4 of the 32 heads end-to-end.
Unlike the absorbed (decode-style) formulation, this kernel materializes
per-head K = kv_c @ wbk^T [S, 128] and V = kv_c @ wbv [S, 128] explicitly,
so scores contract over 192 dims (128 nope + 64 rope) instead of 576 and
the attention output contracts over 128 instead of 512 — ~1.6x fewer MACs.

The kv_a projection + rms-norm + rope (otherwise replicated on all 8 cores)
is sharded over the sequence: each core computes 2 of the 16 kv tiles,
transposes them, and an AllGather collective distributes the transposed
latents while the PE runs the q projections.

All matmul operands are bf16 (1 cycle/row on the PE, same as f32r, but half
the SBUF/DMA traffic); softmax statistics and rope stay f32. Per-core
partial outputs of the row-parallel wo matmul are summed on the host.

Self-contained: hardcodes all shapes from the problem spec.
"""

import os
from contextlib import ExitStack

import numpy as np

import concourse.bacc as bacc
import concourse.bass as bass
import concourse.mybir as mybir
import concourse.tile as tile
from concourse.bass_utils import run_bass_kernel_spmd
from concourse.masks import make_identity

# ---- problem constants ----
DIM = 2048
NH = 32
DN = 128  # qk_nope_head_dim
DR = 64   # qk_rope_head_dim
DV = 128  # v_head_dim
KVL = 512  # kv_lora_rank
S = 2048   # sequence length (B=1)
SCALE = float((DN + DR) ** -0.5)
EPS = 1e-6

NCORES = 8
NHC = NH // NCORES      # heads per core = 4
P = 128                 # partitions
SF = 512                # free-dim tile (s tiles)
NST = S // SF           # 4 s tiles
NTT = S // P            # 16 t tiles
NDC = DIM // P          # 16 contraction chunks over model dim
NCC = KVL // P          # 4 latent chunks

F32 = mybir.dt.float32
BF16 = mybir.dt.bfloat16
F16 = mybir.dt.float16
RT = BF16  # dtype for all matmul operands

# Shard the kv_a projection across cores + AllGather (vs replicate)
SHARD_KV = os.environ.get("MLA_SHARD_KV", "1") == "1"
MSP_BUFS = int(os.environ.get("MLA_MSP_BUFS", "4"))
OHP_BUFS = int(os.environ.get("MLA_OHP_BUFS", "2"))
ETP_BUFS = int(os.environ.get("MLA_ETP_BUFS", "6"))
TSH = S // NCORES  # 256 seq positions (2 t tiles) owned per core


def build_nc(repeat=1):
    """Build the per-core Bass program (identical on all 8 cores)."""
    nc = bacc.Bacc("TRN2", target_bir_lowering=False, debug=False,
                   num_devices=NCORES)

    # ---- DRAM I/O ----
    d_xT = nc.dram_tensor("xT", [DIM, S], RT, kind="ExternalInput")
    d_wqn = nc.dram_tensor("wq_n", [DIM, NHC * DN], RT, kind="ExternalInput")
    d_wqpr = nc.dram_tensor("wq_pr", [DIM, NHC * 32], RT, kind="ExternalInput")
    d_wqpi = nc.dram_tensor("wq_pi", [DIM, NHC * 32], RT, kind="ExternalInput")
    d_wkva = nc.dram_tensor("wkv_a", [DIM, KVL + DR], RT, kind="ExternalInput")
    d_wbkT = nc.dram_tensor("wbkT", [NHC, KVL, DN], RT, kind="ExternalInput")
    d_wbv = nc.dram_tensor("wbv_all", [KVL, NHC * DV], RT,
                           kind="ExternalInput")
    d_wo = nc.dram_tensor("wo_c", [NHC * DV, DIM], RT, kind="ExternalInput")
    d_cosr = nc.dram_tensor("cosR", [P, S], F32, kind="ExternalInput")
    d_sinr = nc.dram_tensor("sinR", [P, S], F32, kind="ExternalInput")
    if SHARD_KV:
        d_xo = nc.dram_tensor("x_own", [DIM, TSH], RT, kind="ExternalInput")
        d_coso = nc.dram_tensor("cos_o", [P, 2 * 32], F32,
                                kind="ExternalInput")
        d_sino = nc.dram_tensor("sin_o", [P, 2 * 32], F32,
                                kind="ExternalInput")
    else:
        d_cosn = nc.dram_tensor("cos_n", [S, DR // 2], F32,
                                kind="ExternalInput")
        d_sinn = nc.dram_tensor("sin_n", [S, DR // 2], F32,
                                kind="ExternalInput")
    d_out = nc.dram_tensor("outT", [DIM, S], F16, kind="ExternalOutput")

    out = d_out.ap()

    with tile.TileContext(nc) as tc:
      for _rep in range(repeat):
        with ExitStack() as top:
            cst = top.enter_context(tc.tile_pool(name="const", bufs=1))
            ident = cst.tile([P, P], RT, tag="ident", name="ident")
            make_identity(nc, ident[:])
            ones_c = cst.tile([P, 1], RT, tag="ones_c", name="ones_c")
            nc.gpsimd.memset(ones_c[:], 1.0)
            epsb = cst.tile([P, 1], F32, tag="epsb", name="epsb")
            nc.gpsimd.memset(epsb[:], EPS)

            # transposed latents, shared by all heads
            kvtp = top.enter_context(tc.tile_pool(name="kvT", bufs=NCC))
            kptp = top.enter_context(tc.tile_pool(name="kpT", bufs=1))
            kvcT = [kvtp.tile([P, S], RT, tag="kvcT", name="kvcT")
                    for _ in range(NCC)]
            kpeT = kptp.tile([DR, S], RT, tag="kpeT", name="kpeT")
            # q for all 4 heads, kept in SBUF
            qnp = top.enter_context(tc.tile_pool(name="qn", bufs=NHC))
            qns = [qnp.tile([DN, S], RT, tag="qn", name="qn")
                   for _ in range(NHC)]
            qpp = top.enter_context(tc.tile_pool(name="qp", bufs=NHC))
            qps = [qpp.tile([DR, S], RT, tag="qp", name="qp")
                   for _ in range(NHC)]
            wkp = top.enter_context(tc.tile_pool(name="wkva", bufs=1))
            wkva_a = wkp.tile([P, NDC * (KVL + DR)], RT, tag="wkva",
                              name="wkva")

            # ===== phase 0: kv shard (2 t-tiles) + AllGather ===============
            if SHARD_KV:
              with ExitStack() as ph0:
                p0s = ph0.enter_context(tc.tile_pool(name="p0s", bufs=1))
                xo = p0s.tile([P, NDC * TSH], RT, tag="xo", name="xo")
                # interleave x-shard and wkva pieces in consumption
                # order; small pieces spread across more DMA engines
                for q8 in range(8):
                    hd = slice(q8 * (NDC // 8), (q8 + 1) * (NDC // 8))
                    nc.sync.dma_start(
                        xo[:].rearrange("p (d f) -> p d f", d=NDC)[:, hd],
                        d_xo.ap().rearrange("(d p) f -> p d f", p=P)[:, hd])
                    nc.sync.dma_start(
                        wkva_a[:].rearrange("p (d c) -> p d c",
                                            d=NDC)[:, hd],
                        d_wkva.ap().rearrange("(d p) c -> p d c",
                                              p=P)[:, hd])
                coso = p0s.tile([P, 2 * 32], F32, tag="coso", name="coso")
                sino = p0s.tile([P, 2 * 32], F32, tag="sino", name="sino")
                nc.sync.dma_start(coso[:], d_coso.ap())
                nc.sync.dma_start(sino[:], d_sino.ap())
                shT = p0s.tile([P, 5 * TSH], RT, tag="shT", name="shT")
                kvo = [p0s.tile([P, KVL], RT, tag="kvo", name="kvo")
                       for _ in range(2)]
                kpo = [p0s.tile([P, DR], RT, tag="kpo", name="kpo")
                       for _ in range(2)]
                nrm0 = ph0.enter_context(tc.tile_pool(name="nrm0", bufs=2))
                with tc.tile_pool(name="p0a", bufs=4, space="PSUM") as p0a:
                    for ti in range(2):
                        psc = p0a.tile([P, KVL], F32, tag="acc", name="acc")
                        psp = p0a.tile([P, DR], F32, tag="acc",
                                       name="accp", padded_shape=[P, KVL])
                        for d in range(NDC):
                            xtsl = xo[:, d * TSH + ti * P:
                                      d * TSH + (ti + 1) * P]
                            nc.tensor.matmul(
                                psc[:], xtsl,
                                wkva_a[:, d * (KVL + DR):
                                       d * (KVL + DR) + KVL],
                                start=(d == 0), stop=(d == NDC - 1))
                            nc.tensor.matmul(
                                psp[:], xtsl,
                                wkva_a[:, d * (KVL + DR) + KVL:
                                       (d + 1) * (KVL + DR)],
                                start=(d == 0), stop=(d == NDC - 1))
                        sq = nrm0.tile([P, KVL], F32, tag="sq", name="sq")
                        ss = nrm0.tile([P, 1], F32, tag="ss", name="ss")
                        nc.scalar.activation(
                            sq[:], psc[:],
                            mybir.ActivationFunctionType.Square,
                            accum_out=ss[:])
                        rt_ = nrm0.tile([P, 1], F32, tag="rt", name="rt")
                        nc.scalar.activation(
                            rt_[:], ss[:],
                            mybir.ActivationFunctionType.Sqrt,
                            bias=epsb[:], scale=1.0 / KVL)
                        ri = nrm0.tile([P, 1], F32, tag="ri", name="ri")
                        nc.vector.reciprocal(ri[:], rt_[:])
                        nc.scalar.mul(kvo[ti][:], psc[:], ri[:])
                        # k rope (deinterleave to [r(32) | i(32)])
                        cn = coso[:, ti * 32:(ti + 1) * 32]
                        sn = sino[:, ti * 32:(ti + 1) * 32]
                        pe = psp[:].rearrange("p (k two) -> p k two", two=2)
                        xr = pe[:, :, 0:1].rearrange("p k one -> p (k one)")
                        xi = pe[:, :, 1:2].rearrange("p k one -> p (k one)")
                        m1 = nrm0.tile([P, DR // 2], F32, tag="m1", name="m1")
                        m2 = nrm0.tile([P, DR // 2], F32, tag="m2", name="m2")
                        nc.vector.tensor_mul(m1[:], xr, cn)
                        nc.vector.tensor_mul(m2[:], xi, sn)
                        nc.vector.tensor_sub(kpo[ti][:, 0:32], m1[:], m2[:])
                        nc.vector.tensor_mul(m1[:], xr, sn)
                        nc.vector.tensor_mul(m2[:], xi, cn)
                        nc.vector.tensor_add(kpo[ti][:, 32:64], m1[:], m2[:])
                    # transpose own tiles into shard layout
                    with tc.tile_pool(name="tp0", bufs=4,
                                      space="PSUM") as tp0:
                        for ti in range(2):
                            for cc in range(NCC):
                                tp = tp0.tile([P, P], RT, tag="t", name="t")
                                nc.tensor.transpose(
                                    tp[:],
                                    kvo[ti][:, cc * P:(cc + 1) * P],
                                    ident[:])
                                nc.scalar.copy(
                                    shT[:, cc * TSH + ti * P:
                                        cc * TSH + (ti + 1) * P], tp[:])
                            tp = tp0.tile([P, P], RT, tag="t", name="t")
                            nc.tensor.transpose(tp[0:DR, :], kpo[ti][:],
                                                ident[:])
                            nc.scalar.copy(
                                shT[0:DR, NCC * TSH + ti * P:
                                    NCC * TSH + (ti + 1) * P], tp[0:DR, :])
                # ---- AllGather the transposed shard ----
                drp = top.enter_context(
                    tc.tile_pool(name="dram", bufs=1, space="DRAM"))
                bin_ = drp.tile([KVL + DR, TSH], RT, tag="cc_in",
                                name="cc_in")
                bout = drp.tile([NCORES, KVL + DR, TSH], RT, tag="cc_out",
                                name="cc_out", addr_space="Shared")
                nc.gpsimd.dma_start(
                    bin_[0:KVL, :].rearrange("(cc p) f -> p cc f", p=P),
                    shT[:].rearrange("p (b f) -> p b f", b=5)[:, 0:NCC])
                nc.gpsimd.dma_start(
                    bin_[KVL:KVL + DR, :],
                    shT[0:DR, NCC * TSH:5 * TSH])
                nc.gpsimd.collective_compute(
                    "AllGather", mybir.AluOpType.bypass,
                    replica_groups=[list(range(NCORES))],
                    ins=[bin_[:].opt()],
                    outs=[bout[:].opt()])
                for cc in range(NCC):
                    nc.gpsimd.dma_start(
                        kvcT[cc][:].rearrange("p (g f) -> p g f", g=NCORES),
                        bout[:, cc * P:(cc + 1) * P, :].rearrange(
                            "g p f -> p g f"))
                nc.gpsimd.dma_start(
                    kpeT[:].rearrange("p (g f) -> p g f", g=NCORES),
                    bout[:, KVL:KVL + DR, :].rearrange("g p f -> p g f"))

            # ===== phase 1: q (+ kv if not sharded) projections ============
            kvc = kpe = None
            if not SHARD_KV:
                for q4 in range(4):
                    hd = slice(q4 * (NDC // 4), (q4 + 1) * (NDC // 4))
                    nc.sync.dma_start(
                        wkva_a[:].rearrange("p (d c) -> p d c",
                                            d=NDC)[:, hd],
                        d_wkva.ap().rearrange("(d p) c -> p d c",
                                              p=P)[:, hd])
                kvp = top.enter_context(tc.tile_pool(name="kv", bufs=NTT))
                kvc = [kvp.tile([P, KVL], RT, tag="kvc", name="kvc")
                       for _ in range(NTT)]
                kpp = top.enter_context(tc.tile_pool(name="kpe", bufs=NTT))
                kpe = [kpp.tile([P, DR], RT, tag="kpe", name="kpe")
                       for _ in range(NTT)]

            with ExitStack() as ph1:
                wrp = ph1.enter_context(tc.tile_pool(name="wres", bufs=1))
                xsl = ph1.enter_context(tc.tile_pool(name="xsl", bufs=6))
                xTj0 = d_xT.ap()[:, 0:SF].rearrange("(d p) f -> p d f", p=P)
                xh0 = [xsl.tile([P, 4 * SF], RT, tag="xsl", name="xsl")
                       for _ in range(4)]
                wqn_a = wrp.tile([P, NDC * NHC * DN], RT, tag="wqn",
                                 name="wqn")
                wqpr_a = wrp.tile([P, NDC * NHC * 32], RT, tag="wqpr",
                                  name="wqpr")
                wqpi_a = wrp.tile([P, NDC * NHC * 32], RT, tag="wqpi",
                                  name="wqpi")
                for q8 in range(8):
                    hd = slice(q8 * (NDC // 8), (q8 + 1) * (NDC // 8))
                    nc.sync.dma_start(
                        xh0[q8 // 2][:].rearrange(
                            "p (d f) -> p d f", d=4)[:, (q8 % 2) * 2:
                                                     (q8 % 2) * 2 + 2],
                        xTj0[:, 2 * q8:2 * (q8 + 1)])
                    nc.sync.dma_start(
                        wqn_a[:].rearrange("p (d c) -> p d c", d=NDC)[:, hd],
                        d_wqn.ap().rearrange("(d p) c -> p d c", p=P)[:, hd])
                    nc.sync.dma_start(
                        wqpr_a[:].rearrange("p (d c) -> p d c", d=NDC)[:, hd],
                        d_wqpr.ap().rearrange("(d p) c -> p d c", p=P)[:, hd])
                    nc.sync.dma_start(
                        wqpi_a[:].rearrange("p (d c) -> p d c", d=NDC)[:, hd],
                        d_wqpi.ap().rearrange("(d p) c -> p d c", p=P)[:, hd])
                if not SHARD_KV:
                    cna = wrp.tile([P, NTT * 32], F32, tag="cna", name="cna")
                    sna = wrp.tile([P, NTT * 32], F32, tag="sna", name="sna")
                    nc.sync.dma_start(
                        cna[:].rearrange("p (t k) -> p t k", t=NTT),
                        d_cosn.ap().rearrange("(t p) k -> p t k", p=P))
                    nc.sync.dma_start(
                        sna[:].rearrange("p (t k) -> p t k", t=NTT),
                        d_sinn.ap().rearrange("(t p) k -> p t k", p=P))

                rts = ph1.enter_context(tc.tile_pool(name="ropetmp", bufs=1))
                sqs = ph1.enter_context(tc.tile_pool(name="sqs", bufs=2))
                crs = ph1.enter_context(tc.tile_pool(name="crs", bufs=2))
                kct = ph1.enter_context(tc.tile_pool(name="kct", bufs=2))
                nrm = ph1.enter_context(tc.tile_pool(name="nrm", bufs=4))

                with tc.tile_pool(name="acc1", bufs=8, space="PSUM") as qac:
                    for j in range(NST):
                        js = slice(j * SF, (j + 1) * SF)
                        xTj = d_xT.ap()[:, js].rearrange(
                            "(d p) f -> p d f", p=P)
                        if j == 0:
                            xh = xh0
                        else:
                            xh = [xsl.tile([P, 4 * SF], RT, tag="xsl",
                                           name="xsl") for _ in range(4)]
                            for q8 in range(8):
                                nc.sync.dma_start(
                                    xh[q8 // 2][:].rearrange(
                                        "p (d f) -> p d f",
                                        d=4)[:, (q8 % 2) * 2:
                                             (q8 % 2) * 2 + 2],
                                    xTj[:, 2 * q8:2 * (q8 + 1)])
                        # ---- q projections for this s block ----
                        pss = [qac.tile([P, SF], F32, tag="acc", name="acc")
                               for _ in range(NHC + 2)]
                        for d in range(NDC):
                            xs = xh[d // 4][:, (d % 4) * SF:(d % 4 + 1) * SF]
                            for h in range(NHC):
                                nc.tensor.matmul(
                                    pss[h][:],
                                    wqn_a[:, d * NHC * DN + h * DN:
                                          d * NHC * DN + (h + 1) * DN],
                                    xs,
                                    start=(d == 0), stop=(d == NDC - 1))
                            nc.tensor.matmul(
                                pss[NHC][:],
                                wqpr_a[:, d * P:(d + 1) * P], xs,
                                start=(d == 0), stop=(d == NDC - 1))
                            nc.tensor.matmul(
                                pss[NHC + 1][:],
                                wqpi_a[:, d * P:(d + 1) * P], xs,
                                start=(d == 0), stop=(d == NDC - 1))
                        for h in range(NHC):
                            nc.scalar.copy(qns[h][:, js], pss[h][:])
                        # rope rotation for q_pe (even=r, odd=i) off PSUM
                        t1 = rts.tile([P, SF], F32, tag="t1", name="t1")
                        t2 = rts.tile([P, SF], F32, tag="t2", name="t2")
                        ror = rts.tile([P, SF], F32, tag="ror", name="ror")
                        roi = rts.tile([P, SF], F32, tag="roi", name="roi")
                        cR = crs.tile([P, SF], F32, tag="cR", name="cR")
                        sR = crs.tile([P, SF], F32, tag="sR", name="sR")
                        nc.sync.dma_start(cR[:], d_cosr.ap()[:, js])
                        nc.sync.dma_start(sR[:], d_sinr.ap()[:, js])
                        nc.vector.tensor_mul(t1[:], pss[NHC][:], cR[:])
                        nc.vector.tensor_mul(t2[:], pss[NHC + 1][:], sR[:])
                        nc.vector.tensor_sub(ror[:], t1[:], t2[:])
                        nc.vector.tensor_mul(t1[:], pss[NHC][:], sR[:])
                        nc.vector.tensor_mul(t2[:], pss[NHC + 1][:], cR[:])
                        nc.vector.tensor_add(roi[:], t1[:], t2[:])
                        for h in range(NHC):
                            hs = slice(h * 32, (h + 1) * 32)
                            nc.vector.tensor_copy(
                                qps[h][0:32, js], ror[hs, :])
                            nc.vector.tensor_copy(
                                qps[h][32:64, js], roi[hs, :])
                        if SHARD_KV:
                            continue
                        # ---- kv projection for this t block (same x) ----
                        for ti in range(4):
                            psc = qac.tile([P, KVL], F32, tag="acc",
                                           name="acc")
                            psp = qac.tile([P, DR], F32, tag="acc",
                                           name="accp", padded_shape=[P, SF])
                            for d in range(NDC):
                                xs = xh[d // 4]
                                xtsl = xs[:, (d % 4) * SF + ti * P:
                                          (d % 4) * SF + (ti + 1) * P]
                                nc.tensor.matmul(
                                    psc[:], xtsl,
                                    wkva_a[:, d * (KVL + DR):
                                           d * (KVL + DR) + KVL],
                                    start=(d == 0), stop=(d == NDC - 1))
                                nc.tensor.matmul(
                                    psp[:], xtsl,
                                    wkva_a[:, d * (KVL + DR) + KVL:
                                           (d + 1) * (KVL + DR)],
                                    start=(d == 0), stop=(d == NDC - 1))
                            t = j * 4 + ti
                            sq = sqs.tile([P, KVL], F32, tag="sq", name="sq")
                            ss = nrm.tile([P, 1], F32, tag="ss", name="ss")
                            nc.scalar.activation(
                                sq[:], psc[:],
                                mybir.ActivationFunctionType.Square,
                                accum_out=ss[:])
                            rt_ = nrm.tile([P, 1], F32, tag="rt", name="rt")
                            nc.scalar.activation(
                                rt_[:], ss[:],
                                mybir.ActivationFunctionType.Sqrt,
                                bias=epsb[:], scale=1.0 / KVL)
                            ri = nrm.tile([P, 1], F32, tag="ri", name="ri")
                            nc.vector.reciprocal(ri[:], rt_[:])
                            nc.scalar.mul(kvc[t][:], psc[:], ri[:])
                            # k rope (deinterleave to [r(32) | i(32)])
                            cn = cna[:, t * 32:(t + 1) * 32]
                            sn = sna[:, t * 32:(t + 1) * 32]
                            pe = psp[:].rearrange(
                                "p (k two) -> p k two", two=2)
                            xr = pe[:, :, 0:1].rearrange(
                                "p k one -> p (k one)")
                            xi = pe[:, :, 1:2].rearrange(
                                "p k one -> p (k one)")
                            m1 = kct.tile([P, DR // 2], F32, tag="m1",
                                          name="m1")
                            m2 = kct.tile([P, DR // 2], F32, tag="m2",
                                          name="m2")
                            nc.vector.tensor_mul(m1[:], xr, cn)
                            nc.vector.tensor_mul(m2[:], xi, sn)
                            nc.vector.tensor_sub(kpe[t][:, 0:32], m1[:],
                                                 m2[:])
                            nc.vector.tensor_mul(m1[:], xr, sn)
                            nc.vector.tensor_mul(m2[:], xi, cn)
                            nc.vector.tensor_add(kpe[t][:, 32:64], m1[:],
                                                 m2[:])

            # ======== phase 2: transposes, K/V materialize, attention ======
            msp = top.enter_context(
                tc.tile_pool(name="msp", bufs=MSP_BUFS, space="PSUM"))
            otp = top.enter_context(tc.tile_pool(name="oT", bufs=NHC))
            oTs = [otp.tile([DV, S], RT, tag="oT", name="oT")
                   for _ in range(NHC)]
            if not SHARD_KV:
                with tc.tile_pool(name="tpp", bufs=4, space="PSUM") as tpp:
                    for t in range(NTT):
                        ts_ = slice(t * P, (t + 1) * P)
                        for cc in range(NCC):
                            tp = tpp.tile([P, P], RT, tag="mspt",
                                          name="mspt")
                            nc.tensor.transpose(
                                tp[:], kvc[t][:, cc * P:(cc + 1) * P],
                                ident[:])
                            nc.scalar.copy(kvcT[cc][:, ts_], tp[:])
                        tp = tpp.tile([P, P], RT, tag="mspt", name="mspt")
                        nc.tensor.transpose(tp[0:DR, :], kpe[t][:], ident[:])
                        nc.scalar.copy(kpeT[:, ts_], tp[0:DR, :])

            # V for all 4 heads: V_all[t] = kv_c[t] @ wbv  -> [t(128), 4*DV]
            vap = top.enter_context(tc.tile_pool(name="vall", bufs=NTT))
            wbvp = top.enter_context(tc.tile_pool(name="wbv", bufs=1))
            wbv_a = wbvp.tile([P, NCC * NHC * DV], RT, tag="wbv",
                              name="wbv")
            nc.sync.dma_start(
                wbv_a[:].rearrange("p (cc f) -> p cc f", cc=NCC),
                d_wbv.ap().rearrange("(cc p) f -> p cc f", p=P))
            vall = [vap.tile([P, NHC * DV], RT, tag="vall", name="vall")
                    for _ in range(NTT)]
            for t in range(NTT):
                ts_ = slice(t * P, (t + 1) * P)
                ps = msp.tile([P, SF], F32, tag="msp", name="msp")
                for cc in range(NCC):
                    nc.tensor.matmul(
                        ps[:], kvcT[cc][:, ts_],
                        wbv_a[:, cc * NHC * DV:(cc + 1) * NHC * DV],
                        start=(cc == 0), stop=(cc == NCC - 1))
                nc.scalar.copy(vall[t][:], ps[:])

            with ExitStack() as ph2:
                wbkp = ph2.enter_context(tc.tile_pool(name="wbk", bufs=2))
                ktp = ph2.enter_context(tc.tile_pool(name="kt", bufs=NHC))
                etp = ph2.enter_context(tc.tile_pool(name="et", bufs=ETP_BUFS))
                ohp = ph2.enter_context(
                    tc.tile_pool(name="ohp", bufs=OHP_BUFS, space="PSUM"))
                dnp = ph2.enter_context(
                    tc.tile_pool(name="dn", bufs=2, space="PSUM"))
                dvp = ph2.enter_context(tc.tile_pool(name="dinv", bufs=2))
                wop = ph2.enter_context(tc.tile_pool(name="wo", bufs=NHC))
                otg = ph2.enter_context(tc.tile_pool(name="ost", bufs=3))

                # prefetch wo weights early (consumed by the per-j output
                # projection below)
                wos = [wop.tile([DV, DIM], RT, tag="wo", name="wo")
                       for _ in range(NHC)]
                for h in range(NHC):
                    nc.sync.dma_start(
                        wos[h][:], d_wo.ap()[h * DV:(h + 1) * DV, :])

                # K_h^T = wbk_h^T(scaled) @ kv_c^T   [DN, S], all heads
                kts = [ktp.tile([DN, S], RT, tag="kT", name="kT")
                       for _ in range(NHC)]
                for h in range(NHC):
                    wbk = wbkp.tile([P, NCC * DN], RT, tag="wbk", name="wbk")
                    nc.sync.dma_start(
                        wbk[:].rearrange("p (cc f) -> p cc f", cc=NCC),
                        d_wbkT.ap()[h].rearrange("(cc p) f -> p cc f", p=P))
                    for j in range(NST):
                        js = slice(j * SF, (j + 1) * SF)
                        ps = msp.tile([P, SF], F32, tag="msp", name="msp")
                        for cc in range(NCC):
                            nc.tensor.matmul(
                                ps[:], wbk[:, cc * DN:(cc + 1) * DN],
                                kvcT[cc][:, js],
                                start=(cc == 0), stop=(cc == NCC - 1))
                        nc.scalar.copy(kts[h][:, js], ps[:])

                # attention j-outer / h-inner (heads interleave to hide the
                # exp->mask latency on diagonal tiles), then the output
                # projection for this j right away
                for j in range(NST):
                    js = slice(j * SF, (j + 1) * SF)
                    for h in range(NHC):
                        oh = ohp.tile([P, SF], F32, tag="oh", name="oh")
                        dn = dnp.tile([1, SF], F32, tag="dn", name="dn")
                        ntt = 4 * j + 4
                        for t in range(ntt):
                            ts_ = slice(t * P, (t + 1) * P)
                            # causal narrowing: tile t only needs columns
                            # s >= t*128, i.e. local offset 128*(t-4j)
                            off = max(0, P * (t - 4 * j))
                            nf = SF - off
                            osl = slice(j * SF + off, (j + 1) * SF)
                            sc = msp.tile([P, SF], F32, tag="msp",
                                          name="msp")
                            nc.tensor.matmul(
                                sc[:, 0:nf], kts[h][:, ts_], qns[h][:, osl],
                                start=True, stop=False)
                            nc.tensor.matmul(
                                sc[:, 0:nf], kpeT[:, ts_], qps[h][:, osl],
                                start=False, stop=True)
                            e = etp.tile([P, SF], RT, tag="et", name="et")
                            nc.scalar.activation(
                                e[:, 0:nf], sc[:, 0:nf],
                                mybir.ActivationFunctionType.Exp,
                                scale=SCALE)
                            if t >= 4 * j:
                                # triangular mask on the first 128 columns
                                # of the diagonal tile (keep c >= p)
                                nc.gpsimd.affine_select(
                                    out=e[:, 0:P], in_=e[:, 0:P],
                                    compare_op=mybir.AluOpType.is_ge,
                                    fill=0.0, base=0,
                                    pattern=[[1, P]],
                                    channel_multiplier=-1)
                            nc.tensor.matmul(
                                dn[:, off:SF], ones_c[:], e[:, 0:nf],
                                start=(t == 0), stop=(t == ntt - 1))
                            nc.tensor.matmul(
                                oh[:, off:SF],
                                vall[t][:, h * DV:(h + 1) * DV],
                                e[:, 0:nf], start=(t == 0),
                                stop=(t == ntt - 1))
                        di = dvp.tile([1, SF], RT, tag="di", name="di")
                        with nc.allow_low_precision(
                                reason="bf16 1/denom is within tolerance"):
                            nc.vector.reciprocal(di[:], dn[:])
                        db = dvp.tile([P, SF], RT, tag="db", name="db")
                        nc.gpsimd.partition_broadcast(db[:], di[:])
                        nc.vector.tensor_mul(oTs[h][:, js], oh[:], db[:])
                    # ---- output projection for this s block ----
                    for d in range(NDC):
                        ds_ = slice(d * P, (d + 1) * P)
                        ps = msp.tile([P, SF], F32, tag="msp", name="msp")
                        for h in range(NHC):
                            nc.tensor.matmul(
                                ps[:], wos[h][:, ds_], oTs[h][:, js],
                                start=(h == 0), stop=(h == NHC - 1))
                        obig = otg.tile([P, SF], F16, tag="ost", name="ost")
                        nc.vector.tensor_copy(obig[:], ps[:])
                        # out DMAs go on the ACT hwdge queue: they wait on
                        # compute, and would head-of-line block the next
                        # rep's input loads on the SP queue
                        nc.scalar.dma_start(out[ds_, js], obig[:])

    nc.compile()
    return nc


def prep_inputs(x, wq_w, wkv_a_w, wkv_b_w, kv_norm_w, wo_w,
                freqs_cos, freqs_sin):
    """Host-side sharding/layout prep. Returns per-core input maps."""
    import ml_dtypes
    bf16 = ml_dtypes.bfloat16
    x = np.ascontiguousarray(np.asarray(x, np.float32).reshape(S, DIM))
    xT = np.ascontiguousarray(x.T).astype(bf16)
    wq = np.asarray(wq_w, np.float32).reshape(DIM, NH, DN + DR)
    wkva = np.ascontiguousarray(np.asarray(wkv_a_w, np.float32)).astype(bf16)
    wkvb = np.asarray(wkv_b_w, np.float32)
    knw = np.asarray(kv_norm_w, np.float32)
    wo = np.asarray(wo_w, np.float32)
    cos = np.asarray(freqs_cos, np.float32)
    sin = np.asarray(freqs_sin, np.float32)
    cosR = np.ascontiguousarray(np.tile(cos.T, (NHC, 1)))  # [128, S]
    sinR = np.ascontiguousarray(np.tile(sin.T, (NHC, 1)))

    maps = []
    for c in range(NCORES):
        hs = list(range(NHC * c, NHC * (c + 1)))
        wq_n = np.ascontiguousarray(
            wq[:, hs, :DN].reshape(DIM, NHC * DN)).astype(bf16)
        wq_pr = np.ascontiguousarray(
            wq[:, hs, DN + 0::2].reshape(DIM, NHC * 32)).astype(bf16)
        wq_pi = np.ascontiguousarray(
            wq[:, hs, DN + 1::2].reshape(DIM, NHC * 32)).astype(bf16)
        # fold kv_norm weight into the absorbed weights
        # wbkT[h] = (wbk_h * knw).T  [KVL, DN]
        wbkT = np.stack([
            np.ascontiguousarray(
                (wkvb[h * (DN + DV):h * (DN + DV) + DN, :] * knw[None, :]).T)
            for h in hs]).astype(bf16)                      # [4, 512, 128]
        # wbv_all = concat_h (wbv_h^T * knw[:,None])  [KVL, 4*DV]
        wbv_all = np.concatenate(
            [np.ascontiguousarray(
                wkvb[h * (DN + DV) + DN:(h + 1) * (DN + DV), :].T)
             * knw[:, None] for h in hs], axis=1).astype(bf16)  # [512, 512]
        wo_c = np.ascontiguousarray(
            np.concatenate([wo[h * DV:(h + 1) * DV, :]
                            for h in hs])).astype(bf16)
        m = {
            "xT": xT, "wq_n": wq_n, "wq_pr": wq_pr, "wq_pi": wq_pi,
            "wkv_a": wkva, "wbkT": np.ascontiguousarray(wbkT),
            "wbv_all": np.ascontiguousarray(wbv_all), "wo_c": wo_c,
            "cosR": cosR, "sinR": sinR,
        }
        if SHARD_KV:
            m["x_own"] = np.ascontiguousarray(
                xT[:, c * TSH:(c + 1) * TSH])
            # cos/sin for own 2 t-tiles, [128, 2*32]
            co = cos[c * TSH:(c + 1) * TSH].reshape(2, P, DR // 2)
            si = sin[c * TSH:(c + 1) * TSH].reshape(2, P, DR // 2)
            m["cos_o"] = np.ascontiguousarray(
                co.transpose(1, 0, 2).reshape(P, 2 * 32))
            m["sin_o"] = np.ascontiguousarray(
                si.transpose(1, 0, 2).reshape(P, 2 * 32))
        else:
            m["cos_n"] = cos
            m["sin_n"] = sin
        maps.append(m)
    return maps


def kernel(x, wq_w, wkv_a_w, wkv_b_w, kv_norm_w, wo_w,
           freqs_cos, freqs_sin, start_pos):
    assert int(start_pos) == 0
    maps = prep_inputs(x, wq_w, wkv_a_w, wkv_b_w, kv_norm_w, wo_w,
                       freqs_cos, freqs_sin)
    nc = build_nc()
    res = run_bass_kernel_spmd(nc, maps, list(range(NCORES)))
    acc = np.zeros((DIM, S), np.float64)
    for c in range(NCORES):
        acc += res.results[c]["outT"].astype(np.float64)
    return np.ascontiguousarray(acc.T).astype(np.float32).reshape(1, S, DIM)


# revision 26
# speedup vs baseline: 1.2411x; 1.1372x over previous
"""MLA (multi-head latent attention) prefill block on 8 Trainium2 NeuronCores.

Tensor-parallel over heads: each core computes 4 of the 32 heads end-to-end.
Unlike the absorbed (decode-style) formulation, this kernel materializes
per-head K = kv_c @ wbk^T [S, 128] and V = kv_c @ wbv [S, 128] explicitly,
so scores contract over 192 dims (128 nope + 64 rope) instead of 576 and
the attention output contracts over 128 instead of 512 — ~1.6x fewer MACs.

The kv_a projection + rms-norm + rope (otherwise replicated on all 8 cores)
is sharded over the sequence: each core computes 2 of the 16 kv tiles,
transposes them, and an AllGather collective distributes the transposed
latents while the PE runs the q projections.

All matmul operands are bf16 (1 cycle/row on the PE, same as f32r, but half
the SBUF/DMA traffic); softmax statistics and rope stay f32. Per-core
partial outputs of the row-parallel wo matmul are summed on the host.

Self-contained: hardcodes all shapes from the problem spec.
"""

import os
from contextlib import ExitStack

import numpy as np

import concourse.bacc as bacc
import concourse.bass as bass
import concourse.mybir as mybir
import concourse.tile as tile
from concourse.bass_utils import run_bass_kernel_spmd
from concourse.masks import make_identity

# ---- problem constants ----
DIM = 2048
NH = 32
DN = 128  # qk_nope_head_dim
DR = 64   # qk_rope_head_dim
DV = 128  # v_head_dim
KVL = 512  # kv_lora_rank
S = 2048   # sequence length (B=1)
SCALE = float((DN + DR) ** -0.5)
EPS = 1e-6

NCORES = 8
NHC = NH // NCORES      # heads per core = 4
P = 128                 # partitions
SF = 512                # free-dim tile (s tiles)
NST = S // SF           # 4 s tiles
NTT = S // P            # 16 t tiles
NDC = DIM // P          # 16 contraction chunks over model dim
NCC = KVL // P          # 4 latent chunks

F32 = mybir.dt.float32
BF16 = mybir.dt.bfloat16
F16 = mybir.dt.float16
RT = BF16  # dtype for all matmul operands

# Shard the kv_a projection across cores + AllGather (vs replicate)
SHARD_KV = os.environ.get("MLA_SHARD_KV", "1") == "1"
MSP_BUFS = int(os.environ.get("MLA_MSP_BUFS", "4"))
OHP_BUFS = int(os.environ.get("MLA_OHP_BUFS", "2"))
ETP_BUFS = int(os.environ.get("MLA_ETP_BUFS", "6"))
TSH = S // NCORES  # 256 seq positions (2 t tiles) owned per core


def build_nc(repeat=1):
    """Build the per-core Bass program (identical on all 8 cores)."""
    nc = bacc.Bacc("TRN2", target_bir_lowering=False, debug=False,
                   num_devices=NCORES)

    # ---- DRAM I/O ----
    d_xT = nc.dram_tensor("xT", [DIM, S], RT, kind="ExternalInput")
    d_wqn = nc.dram_tensor("wq_n", [DIM, NHC * DN], RT, kind="ExternalInput")
    d_wqpr = nc.dram_tensor("wq_pr", [DIM, NHC * 32], RT, kind="ExternalInput")
    d_wqpi = nc.dram_tensor("wq_pi", [DIM, NHC * 32], RT, kind="ExternalInput")
    d_wkva = nc.dram_tensor("wkv_a", [DIM, KVL + DR], RT, kind="ExternalInput")
    d_wbkT = nc.dram_tensor("wbkT", [NHC, KVL, DN], RT, kind="ExternalInput")
    d_wbv = nc.dram_tensor("wbv_all", [KVL, NHC * DV], RT,
                           kind="ExternalInput")
    d_wo = nc.dram_tensor("wo_c", [NHC * DV, DIM], RT, kind="ExternalInput")
    d_cosr = nc.dram_tensor("cosR", [P, S], F32, kind="ExternalInput")
    d_sinr = nc.dram_tensor("sinR", [P, S], F32, kind="ExternalInput")
    if SHARD_KV:
        d_xo = nc.dram_tensor("x_own", [DIM, TSH], RT, kind="ExternalInput")
        d_coso = nc.dram_tensor("cos_o", [P, 2 * 32], F32,
                                kind="ExternalInput")
        d_sino = nc.dram_tensor("sin_o", [P, 2 * 32], F32,
                                kind="ExternalInput")
    else:
        d_cosn = nc.dram_tensor("cos_n", [S, DR // 2], F32,
                                kind="ExternalInput")
        d_sinn = nc.dram_tensor("sin_n", [S, DR // 2], F32,
                                kind="ExternalInput")
    d_out = nc.dram_tensor("outT", [DIM, S], F16, kind="ExternalOutput")

    out = d_out.ap()

    with tile.TileContext(nc) as tc:
      for _rep in range(repeat):
        with ExitStack() as top:
            cst = top.enter_context(tc.tile_pool(name="const", bufs=1))
            ident = cst.tile([P, P], RT, tag="ident", name="ident")
            make_identity(nc, ident[:])
            ones_c = cst.tile([P, 1], RT, tag="ones_c", name="ones_c")
            nc.gpsimd.memset(ones_c[:], 1.0)
            epsb = cst.tile([P, 1], F32, tag="epsb", name="epsb")
            nc.gpsimd.memset(epsb[:], EPS)

            # transposed latents, shared by all heads
            kvtp = top.enter_context(tc.tile_pool(name="kvT", bufs=NCC))
            kptp = top.enter_context(tc.tile_pool(name="kpT", bufs=1))
            kvcT = [kvtp.tile([P, S], RT, tag="kvcT", name="kvcT")
                    for _ in range(NCC)]
            kpeT = kptp.tile([DR, S], RT, tag="kpeT", name="kpeT")
            # q for all 4 heads, kept in SBUF
            qnp = top.enter_context(tc.tile_pool(name="qn", bufs=NHC))
            qns = [qnp.tile([DN, S], RT, tag="qn", name="qn")
                   for _ in range(NHC)]
            qpp = top.enter_context(tc.tile_pool(name="qp", bufs=NHC))
            qps = [qpp.tile([DR, S], RT, tag="qp", name="qp")
                   for _ in range(NHC)]
            wkp = top.enter_context(tc.tile_pool(name="wkva", bufs=1))
            wkva_a = wkp.tile([P, NDC * (KVL + DR)], RT, tag="wkva",
                              name="wkva")

            # ===== phase 0: kv shard (2 t-tiles) + AllGather ===============
            if SHARD_KV:
              with ExitStack() as ph0:
                p0s = ph0.enter_context(tc.tile_pool(name="p0s", bufs=1))
                xo = p0s.tile([P, NDC * TSH], RT, tag="xo", name="xo")
                # interleave x-shard and wkva pieces in consumption
                # order; small pieces spread across more DMA engines
                for q8 in range(8):
                    hd = slice(q8 * (NDC // 8), (q8 + 1) * (NDC // 8))
                    nc.sync.dma_start(
                        xo[:].rearrange("p (d f) -> p d f", d=NDC)[:, hd],
                        d_xo.ap().rearrange("(d p) f -> p d f", p=P)[:, hd])
                    nc.sync.dma_start(
                        wkva_a[:].rearrange("p (d c) -> p d c",
                                            d=NDC)[:, hd],
                        d_wkva.ap().rearrange("(d p) c -> p d c",
                                              p=P)[:, hd])
                coso = p0s.tile([P, 2 * 32], F32, tag="coso", name="coso")
                sino = p0s.tile([P, 2 * 32], F32, tag="sino", name="sino")
                nc.sync.dma_start(coso[:], d_coso.ap())
                nc.sync.dma_start(sino[:], d_sino.ap())
                shT = p0s.tile([P, 5 * TSH], RT, tag="shT", name="shT")
                kvo = [p0s.tile([P, KVL], RT, tag="kvo", name="kvo")
                       for _ in range(2)]
                kpo = [p0s.tile([P, DR], RT, tag="kpo", name="kpo")
                       for _ in range(2)]
                nrm0 = ph0.enter_context(tc.tile_pool(name="nrm0", bufs=2))
                with tc.tile_pool(name="p0a", bufs=4, space="PSUM") as p0a:
                    for ti in range(2):
                        psc = p0a.tile([P, KVL], F32, tag="acc", name="acc")
                        psp = p0a.tile([P, DR], F32, tag="acc",
                                       name="accp", padded_shape=[P, KVL])
                        for d in range(NDC):
                            xtsl = xo[:, d * TSH + ti * P:
                                      d * TSH + (ti + 1) * P]
                            nc.tensor.matmul(
                                psc[:], xtsl,
                                wkva_a[:, d * (KVL + DR):
                                       d * (KVL + DR) + KVL],
                                start=(d == 0), stop=(d == NDC - 1))
                            nc.tensor.matmul(
                                psp[:], xtsl,
                                wkva_a[:, d * (KVL + DR) + KVL:
                                       (d + 1) * (KVL + DR)],
                                start=(d == 0), stop=(d == NDC - 1))
                        sq = nrm0.tile([P, KVL], F32, tag="sq", name="sq")
                        ss = nrm0.tile([P, 1], F32, tag="ss", name="ss")
                        nc.scalar.activation(
                            sq[:], psc[:],
                            mybir.ActivationFunctionType.Square,
                            accum_out=ss[:])
                        rt_ = nrm0.tile([P, 1], F32, tag="rt", name="rt")
                        nc.scalar.activation(
                            rt_[:], ss[:],
                            mybir.ActivationFunctionType.Sqrt,
                            bias=epsb[:], scale=1.0 / KVL)
                        ri = nrm0.tile([P, 1], F32, tag="ri", name="ri")
                        nc.vector.reciprocal(ri[:], rt_[:])
                        nc.scalar.mul(kvo[ti][:], psc[:], ri[:])
                        # k rope (deinterleave to [r(32) | i(32)])
                        cn = coso[:, ti * 32:(ti + 1) * 32]
                        sn = sino[:, ti * 32:(ti + 1) * 32]
                        pe = psp[:].rearrange("p (k two) -> p k two", two=2)
                        xr = pe[:, :, 0:1].rearrange("p k one -> p (k one)")
                        xi = pe[:, :, 1:2].rearrange("p k one -> p (k one)")
                        m1 = nrm0.tile([P, DR // 2], F32, tag="m1", name="m1")
                        m2 = nrm0.tile([P, DR // 2], F32, tag="m2", name="m2")
                        nc.vector.tensor_mul(m1[:], xr, cn)
                        nc.vector.tensor_mul(m2[:], xi, sn)
                        nc.vector.tensor_sub(kpo[ti][:, 0:32], m1[:], m2[:])
                        nc.vector.tensor_mul(m1[:], xr, sn)
                        nc.vector.tensor_mul(m2[:], xi, cn)
                        nc.vector.tensor_add(kpo[ti][:, 32:64], m1[:], m2[:])
                    # transpose own tiles into shard layout
                    with tc.tile_pool(name="tp0", bufs=4,
                                      space="PSUM") as tp0:
                        for ti in range(2):
                            for cc in range(NCC):
                                tp = tp0.tile([P, P], RT, tag="t", name="t")
                                nc.tensor.transpose(
                                    tp[:],
                                    kvo[ti][:, cc * P:(cc + 1) * P],
                                    ident[:])
                                nc.scalar.copy(
                                    shT[:, cc * TSH + ti * P:
                                        cc * TSH + (ti + 1) * P], tp[:])
                            tp = tp0.tile([P, P], RT, tag="t", name="t")
                            nc.tensor.transpose(tp[0:DR, :], kpo[ti][:],
                                                ident[:])
                            nc.scalar.copy(
                                shT[0:DR, NCC * TSH + ti * P:
                                    NCC * TSH + (ti + 1) * P], tp[0:DR, :])
                # ---- AllGather the transposed shard ----
                drp = top.enter_context(
                    tc.tile_pool(name="dram", bufs=1, space="DRAM"))
                bin_ = drp.tile([KVL + DR, TSH], RT, tag="cc_in",
                                name="cc_in")
                bout = drp.tile([NCORES, KVL + DR, TSH], RT, tag="cc_out",
                                name="cc_out", addr_space="Shared")
                nc.gpsimd.dma_start(
                    bin_[0:KVL, :].rearrange("(cc p) f -> p cc f", p=P),
                    shT[:].rearrange("p (b f) -> p b f", b=5)[:, 0:NCC])
                nc.gpsimd.dma_start(
                    bin_[KVL:KVL + DR, :],
                    shT[0:DR, NCC * TSH:5 * TSH])
                nc.gpsimd.collective_compute(
                    "AllGather", mybir.AluOpType.bypass,
                    replica_groups=[list(range(NCORES))],
                    ins=[bin_[:].opt()],
                    outs=[bout[:].opt()])
                for cc in range(NCC):
                    nc.gpsimd.dma_start(
                        kvcT[cc][:].rearrange("p (g f) -> p g f", g=NCORES),
                        bout[:, cc * P:(cc + 1) * P, :].rearrange(
                            "g p f -> p g f"))
                nc.gpsimd.dma_start(
                    kpeT[:].rearrange("p (g f) -> p g f", g=NCORES),
                    bout[:, KVL:KVL + DR, :].rearrange("g p f -> p g f"))

            # ===== phase 1: q (+ kv if not sharded) projections ============
            kvc = kpe = None
            if not SHARD_KV:
                for q4 in range(4):
                    hd = slice(q4 * (NDC // 4), (q4 + 1) * (NDC // 4))
                    nc.sync.dma_start(
                        wkva_a[:].rearrange("p (d c) -> p d c",
                                            d=NDC)[:, hd],
                        d_wkva.ap().rearrange("(d p) c -> p d c",
                                              p=P)[:, hd])
                kvp = top.enter_context(tc.tile_pool(name="kv", bufs=NTT))
                kvc = [kvp.tile([P, KVL], RT, tag="kvc", name="kvc")
                       for _ in range(NTT)]
                kpp = top.enter_context(tc.tile_pool(name="kpe", bufs=NTT))
                kpe = [kpp.tile([P, DR], RT, tag="kpe", name="kpe")
                       for _ in range(NTT)]

            with ExitStack() as ph1:
                wrp = ph1.enter_context(tc.tile_pool(name="wres", bufs=1))
                xsl = ph1.enter_context(tc.tile_pool(name="xsl", bufs=6))
                xTj0 = d_xT.ap()[:, 0:SF].rearrange("(d p) f -> p d f", p=P)
                xh0 = [xsl.tile([P, 4 * SF], RT, tag="xsl", name="xsl")
                       for _ in range(4)]
                wqn_a = wrp.tile([P, NDC * NHC * DN], RT, tag="wqn",
                                 name="wqn")
                wqpr_a = wrp.tile([P, NDC * NHC * 32], RT, tag="wqpr",
                                  name="wqpr")
                wqpi_a = wrp.tile([P, NDC * NHC * 32], RT, tag="wqpi",
                                  name="wqpi")
                for q8 in range(8):
                    hd = slice(q8 * (NDC // 8), (q8 + 1) * (NDC // 8))
                    nc.sync.dma_start(
                        xh0[q8 // 2][:].rearrange(
                            "p (d f) -> p d f", d=4)[:, (q8 % 2) * 2:
                                                     (q8 % 2) * 2 + 2],
                        xTj0[:, 2 * q8:2 * (q8 + 1)])
                    nc.sync.dma_start(
                        wqn_a[:].rearrange("p (d c) -> p d c", d=NDC)[:, hd],
                        d_wqn.ap().rearrange("(d p) c -> p d c", p=P)[:, hd])
                    nc.sync.dma_start(
                        wqpr_a[:].rearrange("p (d c) -> p d c", d=NDC)[:, hd],
                        d_wqpr.ap().rearrange("(d p) c -> p d c", p=P)[:, hd])
                    nc.sync.dma_start(
                        wqpi_a[:].rearrange("p (d c) -> p d c", d=NDC)[:, hd],
                        d_wqpi.ap().rearrange("(d p) c -> p d c", p=P)[:, hd])
                if not SHARD_KV:
                    cna = wrp.tile([P, NTT * 32], F32, tag="cna", name="cna")
                    sna = wrp.tile([P, NTT * 32], F32, tag="sna", name="sna")
                    nc.sync.dma_start(
                        cna[:].rearrange("p (t k) -> p t k", t=NTT),
                        d_cosn.ap().rearrange("(t p) k -> p t k", p=P))
                    nc.sync.dma_start(
                        sna[:].rearrange("p (t k) -> p t k", t=NTT),
                        d_sinn.ap().rearrange("(t p) k -> p t k", p=P))

                rts = ph1.enter_context(tc.tile_pool(name="ropetmp", bufs=1))
                sqs = ph1.enter_context(tc.tile_pool(name="sqs", bufs=2))
                crs = ph1.enter_context(tc.tile_pool(name="crs", bufs=2))
                kct = ph1.enter_context(tc.tile_pool(name="kct", bufs=2))
                nrm = ph1.enter_context(tc.tile_pool(name="nrm", bufs=4))

                with tc.tile_pool(name="acc1", bufs=8, space="PSUM") as qac:
                    for j in range(NST):
                        js = slice(j * SF, (j + 1) * SF)
                        xTj = d_xT.ap()[:, js].rearrange(
                            "(d p) f -> p d f", p=P)
                        if j == 0:
                            xh = xh0
                        else:
                            xh = [xsl.tile([P, 4 * SF], RT, tag="xsl",
                                           name="xsl") for _ in range(4)]
                            for q8 in range(8):
                                nc.sync.dma_start(
                                    xh[q8 // 2][:].rearrange(
                                        "p (d f) -> p d f",
                                        d=4)[:, (q8 % 2) * 2:
                                             (q8 % 2) * 2 + 2],
                                    xTj[:, 2 * q8:2 * (q8 + 1)])
                        # ---- q projections for this s block ----
                        pss = [qac.tile([P, SF], F32, tag="acc", name="acc")
                               for _ in range(NHC + 2)]
                        for d in range(NDC):
                            xs = xh[d // 4][:, (d % 4) * SF:(d % 4 + 1) * SF]
                            for h in range(NHC):
                                nc.tensor.matmul(
                                    pss[h][:],
                                    wqn_a[:, d * NHC * DN + h * DN:
                                          d * NHC * DN + (h + 1) * DN],
                                    xs,
                                    start=(d == 0), stop=(d == NDC - 1))
                            nc.tensor.matmul(
                                pss[NHC][:],
                                wqpr_a[:, d * P:(d + 1) * P], xs,
                                start=(d == 0), stop=(d == NDC - 1))
                            nc.tensor.matmul(
                                pss[NHC + 1][:],
                                wqpi_a[:, d * P:(d + 1) * P], xs,
                                start=(d == 0), stop=(d == NDC - 1))
                        for h in range(NHC):
                            nc.scalar.copy(qns[h][:, js], pss[h][:])
                        # rope rotation for q_pe (even=r, odd=i) off PSUM
                        t1 = rts.tile([P, SF], F32, tag="t1", name="t1")
                        t2 = rts.tile([P, SF], F32, tag="t2", name="t2")
                        ror = rts.tile([P, SF], F32, tag="ror", name="ror")
                        roi = rts.tile([P, SF], F32, tag="roi", name="roi")
                        cR = crs.tile([P, SF], F32, tag="cR", name="cR")
                        sR = crs.tile([P, SF], F32, tag="sR", name="sR")
                        nc.sync.dma_start(cR[:], d_cosr.ap()[:, js])
                        nc.sync.dma_start(sR[:], d_sinr.ap()[:, js])
                        nc.vector.tensor_mul(t1[:], pss[NHC][:], cR[:])
                        nc.vector.tensor_mul(t2[:], pss[NHC + 1][:], sR[:])
                        nc.vector.tensor_sub(ror[:], t1[:], t2[:])
                        nc.vector.tensor_mul(t1[:], pss[NHC][:], sR[:])
                        nc.vector.tensor_mul(t2[:], pss[NHC + 1][:], cR[:])
                        nc.vector.tensor_add(roi[:], t1[:], t2[:])
                        for h in range(NHC):
                            hs = slice(h * 32, (h + 1) * 32)
                            nc.vector.tensor_copy(
                                qps[h][0:32, js], ror[hs, :])
                            nc.vector.tensor_copy(
                                qps[h][32:64, js], roi[hs, :])
                        if SHARD_KV:
                            continue
                        # ---- kv projection for this t block (same x) ----
                        for ti in range(4):
                            psc = qac.tile([P, KVL], F32, tag="acc",
                                           name="acc")
                            psp = qac.tile([P, DR], F32, tag="acc",
                                           name="accp", padded_shape=[P, SF])
                            for d in range(NDC):
                                xs = xh[d // 4]
                                xtsl = xs[:, (d % 4) * SF + ti * P:
                                          (d % 4) * SF + (ti + 1) * P]
                                nc.tensor.matmul(
                                    psc[:], xtsl,
                                    wkva_a[:, d * (KVL + DR):
                                           d * (KVL + DR) + KVL],
                                    start=(d == 0), stop=(d == NDC - 1))
                                nc.tensor.matmul(
                                    psp[:], xtsl,
                                    wkva_a[:, d * (KVL + DR) + KVL:
                                           (d + 1) * (KVL + DR)],
                                    start=(d == 0), stop=(d == NDC - 1))
                            t = j * 4 + ti
                            sq = sqs.tile([P, KVL], F32, tag="sq", name="sq")
                            ss = nrm.tile([P, 1], F32, tag="ss", name="ss")
                            nc.scalar.activation(
                                sq[:], psc[:],
                                mybir.ActivationFunctionType.Square,
                                accum_out=ss[:])
                            rt_ = nrm.tile([P, 1], F32, tag="rt", name="rt")
                            nc.scalar.activation(
                                rt_[:], ss[:],
                                mybir.ActivationFunctionType.Sqrt,
                                bias=epsb[:], scale=1.0 / KVL)
                            ri = nrm.tile([P, 1], F32, tag="ri", name="ri")
                            nc.vector.reciprocal(ri[:], rt_[:])
                            nc.scalar.mul(kvc[t][:], psc[:], ri[:])
                            # k rope (deinterleave to [r(32) | i(32)])
                            cn = cna[:, t * 32:(t + 1) * 32]
                            sn = sna[:, t * 32:(t + 1) * 32]
                            pe = psp[:].rearrange(
                                "p (k two) -> p k two", two=2)
                            xr = pe[:, :, 0:1].rearrange(
                                "p k one -> p (k one)")
                            xi = pe[:, :, 1:2].rearrange(
                                "p k one -> p (k one)")
                            m1 = kct.tile([P, DR // 2], F32, tag="m1",
                                          name="m1")
                            m2 = kct.tile([P, DR // 2], F32, tag="m2",
                                          name="m2")
                            nc.vector.tensor_mul(m1[:], xr, cn)
                            nc.vector.tensor_mul(m2[:], xi, sn)
                            nc.vector.tensor_sub(kpe[t][:, 0:32], m1[:],
                                                 m2[:])
                            nc.vector.tensor_mul(m1[:], xr, sn)
                            nc.vector.tensor_mul(m2[:], xi, cn)
                            nc.vector.tensor_add(kpe[t][:, 32:64], m1[:],
                                                 m2[:])

            # ======== phase 2: transposes, K/V materialize, attention ======
            msp = top.enter_context(
                tc.tile_pool(name="msp", bufs=MSP_BUFS, space="PSUM"))
            otp = top.enter_context(tc.tile_pool(name="oT", bufs=NHC))
            oTs = [otp.tile([DV, S], RT, tag="oT", name="oT")
                   for _ in range(NHC)]
            if not SHARD_KV:
                with tc.tile_pool(name="tpp", bufs=4, space="PSUM") as tpp:
                    for t in range(NTT):
                        ts_ = slice(t * P, (t + 1) * P)
                        for cc in range(NCC):
                            tp = tpp.tile([P, P], RT, tag="mspt",
                                          name="mspt")
                            nc.tensor.transpose(
                                tp[:], kvc[t][:, cc * P:(cc + 1) * P],
                                ident[:])
                            nc.scalar.copy(kvcT[cc][:, ts_], tp[:])
                        tp = tpp.tile([P, P], RT, tag="mspt", name="mspt")
                        nc.tensor.transpose(tp[0:DR, :], kpe[t][:], ident[:])
                        nc.scalar.copy(kpeT[:, ts_], tp[0:DR, :])

            # V for all 4 heads: V_all[t] = kv_c[t] @ wbv  -> [t(128), 4*DV]
            vap = top.enter_context(tc.tile_pool(name="vall", bufs=NTT))
            wbvp = top.enter_context(tc.tile_pool(name="wbv", bufs=1))
            wbv_a = wbvp.tile([P, NCC * NHC * DV], RT, tag="wbv",
                              name="wbv")
            nc.sync.dma_start(
                wbv_a[:].rearrange("p (cc f) -> p cc f", cc=NCC),
                d_wbv.ap().rearrange("(cc p) f -> p cc f", p=P))
            vall = [vap.tile([P, NHC * DV], RT, tag="vall", name="vall")
                    for _ in range(NTT)]
            for t in range(NTT):
                ts_ = slice(t * P, (t + 1) * P)
                ps = msp.tile([P, SF], F32, tag="msp", name="msp")
                for cc in range(NCC):
                    nc.tensor.matmul(
                        ps[:], kvcT[cc][:, ts_],
                        wbv_a[:, cc * NHC * DV:(cc + 1) * NHC * DV],
                        start=(cc == 0), stop=(cc == NCC - 1))
                nc.scalar.copy(vall[t][:], ps[:])

            with ExitStack() as ph2:
                wbkp = ph2.enter_context(tc.tile_pool(name="wbk", bufs=2))
                ktp = ph2.enter_context(tc.tile_pool(name="kt", bufs=NHC))
                etp = ph2.enter_context(tc.tile_pool(name="et", bufs=ETP_BUFS))
                ohp = ph2.enter_context(
                    tc.tile_pool(name="ohp", bufs=OHP_BUFS, space="PSUM"))
                dnp = ph2.enter_context(
                    tc.tile_pool(name="dn", bufs=2, space="PSUM"))
                dvp = ph2.enter_context(tc.tile_pool(name="dinv", bufs=3))
                wop = ph2.enter_context(tc.tile_pool(name="wo", bufs=NHC))
                otg = ph2.enter_context(tc.tile_pool(name="ost", bufs=4))

                # prefetch wo weights early (consumed by the per-j output
                # projection below)
                wos = [wop.tile([DV, DIM], RT, tag="wo", name="wo")
                       for _ in range(NHC)]
                for h in range(NHC):
                    nc.sync.dma_start(
                        wos[h][:], d_wo.ap()[h * DV:(h + 1) * DV, :])

                # K_h^T = wbk_h^T(scaled) @ kv_c^T   [DN, S], all heads
                kts = [ktp.tile([DN, S], RT, tag="kT", name="kT")
                       for _ in range(NHC)]
                for h in range(NHC):
                    wbk = wbkp.tile([P, NCC * DN], RT, tag="wbk", name="wbk")
                    nc.sync.dma_start(
                        wbk[:].rearrange("p (cc f) -> p cc f", cc=NCC),
                        d_wbkT.ap()[h].rearrange("(cc p) f -> p cc f", p=P))
                    for j in range(NST):
                        js = slice(j * SF, (j + 1) * SF)
                        ps = msp.tile([P, SF], F32, tag="msp", name="msp")
                        for cc in range(NCC):
                            nc.tensor.matmul(
                                ps[:], wbk[:, cc * DN:(cc + 1) * DN],
                                kvcT[cc][:, js],
                                start=(cc == 0), stop=(cc == NCC - 1))
                        nc.scalar.copy(kts[h][:, js], ps[:])

                # attention j-outer / h-inner (heads interleave to hide the
                # exp->mask latency on diagonal tiles), then the output
                # projection for this j right away
                for j in range(NST):
                    js = slice(j * SF, (j + 1) * SF)
                    for h in range(NHC):
                        oh = ohp.tile([P, SF], F32, tag="oh", name="oh")
                        dn = dnp.tile([1, SF], F32, tag="dn", name="dn")
                        ntt = 4 * j + 4
                        for t in range(ntt):
                            ts_ = slice(t * P, (t + 1) * P)
                            # causal narrowing: tile t only needs columns
                            # s >= t*128, i.e. local offset 128*(t-4j)
                            off = max(0, P * (t - 4 * j))
                            nf = SF - off
                            osl = slice(j * SF + off, (j + 1) * SF)
                            sc = msp.tile([P, SF], F32, tag="msp",
                                          name="msp")
                            nc.tensor.matmul(
                                sc[:, 0:nf], kts[h][:, ts_], qns[h][:, osl],
                                start=True, stop=False)
                            nc.tensor.matmul(
                                sc[:, 0:nf], kpeT[:, ts_], qps[h][:, osl],
                                start=False, stop=True)
                            e = etp.tile([P, SF], RT, tag="et", name="et")
                            nc.scalar.activation(
                                e[:, 0:nf], sc[:, 0:nf],
                                mybir.ActivationFunctionType.Exp,
                                scale=SCALE)
                            if t >= 4 * j:
                                # triangular mask on the first 128 columns
                                # of the diagonal tile (keep c >= p)
                                nc.gpsimd.affine_select(
                                    out=e[:, 0:P], in_=e[:, 0:P],
                                    compare_op=mybir.AluOpType.is_ge,
                                    fill=0.0, base=0,
                                    pattern=[[1, P]],
                                    channel_multiplier=-1)
                            nc.tensor.matmul(
                                dn[:, off:SF], ones_c[:], e[:, 0:nf],
                                start=(t == 0), stop=(t == ntt - 1))
                            nc.tensor.matmul(
                                oh[:, off:SF],
                                vall[t][:, h * DV:(h + 1) * DV],
                                e[:, 0:nf], start=(t == 0),
                                stop=(t == ntt - 1))
                        di = dvp.tile([1, SF], RT, tag="di", name="di")
                        with nc.allow_low_precision(
                                reason="bf16 1/denom is within tolerance"):
                            nc.vector.reciprocal(di[:], dn[:])
                        db = dvp.tile([P, SF], RT, tag="db", name="db")
                        nc.gpsimd.partition_broadcast(db[:], di[:])
                        nc.vector.tensor_mul(oTs[h][:, js], oh[:], db[:])
                    # ---- output projection for this s block ----
                    for d in range(NDC):
                        ds_ = slice(d * P, (d + 1) * P)
                        ps = msp.tile([P, SF], F32, tag="msp", name="msp")
                        for h in range(NHC):
                            nc.tensor.matmul(
                                ps[:], wos[h][:, ds_], oTs[h][:, js],
                                start=(h == 0), stop=(h == NHC - 1))
                        obig = otg.tile([P, SF], F16, tag="ost", name="ost")
                        nc.vector.tensor_copy(obig[:], ps[:])
                        # out DMAs go on the ACT hwdge queue: they wait on
                        # compute, and would head-of-line block the next
                        # rep's input loads on the SP queue
                        nc.scalar.dma_start(out[ds_, js], obig[:])

    nc.compile()
    return nc


def prep_inputs(x, wq_w, wkv_a_w, wkv_b_w, kv_norm_w, wo_w,
                freqs_cos, freqs_sin):
    """Host-side sharding/layout prep. Returns per-core input maps."""
    import ml_dtypes
    bf16 = ml_dtypes.bfloat16
    x = np.ascontiguousarray(np.asarray(x, np.float32).reshape(S, DIM))
    xT = np.ascontiguousarray(x.T).astype(bf16)
    wq = np.asarray(wq_w, np.float32).reshape(DIM, NH, DN + DR)
    wkva = np.ascontiguousarray(np.asarray(wkv_a_w, np.float32)).astype(bf16)
    wkvb = np.asarray(wkv_b_w, np.float32)
    knw = np.asarray(kv_norm_w, np.float32)
    wo = np.asarray(wo_w, np.float32)
    cos = np.asarray(freqs_cos, np.float32)
    sin = np.asarray(freqs_sin, np.float32)
    cosR = np.ascontiguousarray(np.tile(cos.T, (NHC, 1)))  # [128, S]
    sinR = np.ascontiguousarray(np.tile(sin.T, (NHC, 1)))

    maps = []
    for c in range(NCORES):
        hs = list(range(NHC * c, NHC * (c + 1)))
        wq_n = np.ascontiguousarray(
            wq[:, hs, :DN].reshape(DIM, NHC * DN)).astype(bf16)
        wq_pr = np.ascontiguousarray(
            wq[:, hs, DN + 0::2].reshape(DIM, NHC * 32)).astype(bf16)
        wq_pi = np.ascontiguousarray(
            wq[:, hs, DN + 1::2].reshape(DIM, NHC * 32)).astype(bf16)
        # fold kv_norm weight into the absorbed weights
        # wbkT[h] = (wbk_h * knw).T  [KVL, DN]
        wbkT = np.stack([
            np.ascontiguousarray(
                (wkvb[h * (DN + DV):h * (DN + DV) + DN, :] * knw[None, :]).T)
            for h in hs]).astype(bf16)                      # [4, 512, 128]
        # wbv_all = concat_h (wbv_h^T * knw[:,None])  [KVL, 4*DV]
        wbv_all = np.concatenate(
            [np.ascontiguousarray(
                wkvb[h * (DN + DV) + DN:(h + 1) * (DN + DV), :].T)
             * knw[:, None] for h in hs], axis=1).astype(bf16)  # [512, 512]
        wo_c = np.ascontiguousarray(
            np.concatenate([wo[h * DV:(h + 1) * DV, :]
                            for h in hs])).astype(bf16)
        m = {
            "xT": xT, "wq_n": wq_n, "wq_pr": wq_pr, "wq_pi": wq_pi,
            "wkv_a": wkva, "wbkT": np.ascontiguousarray(wbkT),
            "wbv_all": np.ascontiguousarray(wbv_all), "wo_c": wo_c,
            "cosR": cosR, "sinR": sinR,
        }
        if SHARD_KV:
            m["x_own"] = np.ascontiguousarray(
                xT[:, c * TSH:(c + 1) * TSH])
            # cos/sin for own 2 t-tiles, [128, 2*32]
            co = cos[c * TSH:(c + 1) * TSH].reshape(2, P, DR // 2)
            si = sin[c * TSH:(c + 1) * TSH].reshape(2, P, DR // 2)
            m["cos_o"] = np.ascontiguousarray(
                co.transpose(1, 0, 2).reshape(P, 2 * 32))
            m["sin_o"] = np.ascontiguousarray(
                si.transpose(1, 0, 2).reshape(P, 2 * 32))
        else:
            m["cos_n"] = cos
            m["sin_n"] = sin
        maps.append(m)
    return maps


def kernel(x, wq_w, wkv_a_w, wkv_b_w, kv_norm_w, wo_w,
           freqs_cos, freqs_sin, start_pos):
    assert int(start_pos) == 0
    maps = prep_inputs(x, wq_w, wkv_a_w, wkv_b_w, kv_norm_w, wo_w,
                       freqs_cos, freqs_sin)
    nc = build_nc()
    res = run_bass_kernel_spmd(nc, maps, list(range(NCORES)))
    acc = np.zeros((DIM, S), np.float64)
    for c in range(NCORES):
        acc += res.results[c]["outT"].astype(np.float64)
    return np.ascontiguousarray(acc.T).astype(np.float32).reshape(1, S, DIM)
